# revision 31
# baseline (speedup 1.0000x reference)
"""Trainium2 Bass kernel for PosNegBalanceLoss.

Contract: kernel(**inputs) takes FULL unsharded inputs (pred/target/rand_mat
[131072,40] f32, hard_rand [1,40], pos_prop [40]) and returns the FULL scalar
output, distributing across 8 NeuronCores internally.

Sharding: by class columns (40 classes -> 5 per core). Each core gets
contiguous [5, 131072] slices (host-side transpose), so every per-class
reduction including the rank/argsort step is core-local. The only cross-core
step is an 8-byte AllReduce(max) of (ln_max, -ln_min) over per-class losses.

Math: with x2 = pred*(1-2*target), bce = logaddexp(0,pred) - pred*target
== softplus(x2), and g = |sigmoid(pred)-target| == sigmoid(x2), so the whole
elementwise phase is two DVE ops + two ACT table ops per class.

The rank step (drop the dropout_num smallest-g majority samples per class) is
a per-class threshold search: lockstep bisection with exact counts of
(g_masked < t) via ACT Sign+accum / DVE is_ge+accum; cross-partition count
totals are replicated with a PE double-matmul (acc @ ones -> [5,1];
broadcast-copy -> [5,128]; @ I5 -> [128,5]). Per-class scalars stay
replicated across partitions the whole time, so no partition broadcasts are
needed. A fixed number of bisection rounds leaves ~2e-5 relative error on the
final mean (only elements inside the final bisection interval can be
mis-dropped, each contributing ~2e-7).
"""

import numpy as np
from contextlib import ExitStack

import concourse.bass as bass
import concourse.tile as tile
from concourse import bacc, mybir
from concourse.bass_utils import run_bass_kernel_spmd

F32 = mybir.dt.float32
U8 = mybir.dt.uint8
ALU = mybir.AluOpType
AF = mybir.ActivationFunctionType

B = 131072            # batch (rows)
C = 40                # classes (cols)
NCORES = 8
CL = C // NCORES      # classes per core = 5
P = 128               # sbuf partitions
N = B // P            # 1024 elems per partition per class
BF = float(B)
BIS_ITERS = 9         # bisection rounds
NOUT = 32             # debug-friendly output vector per core

_CACHED = {}


def _bc(ap, n):
    """Broadcast a [p,1] AP along the free dim to [p,n] (stride-0)."""
    return bass.AP(tensor=ap.tensor, offset=ap.offset, ap=[ap.ap[0], [0, n]])


def _body(tc: tile.TileContext, pred_d, targ_d, rand_d, scal_d, out_d):
    nc = tc.nc
    ctx = ExitStack()
    main = ctx.enter_context(tc.tile_pool(name="main", bufs=1))
    tmp3 = ctx.enter_context(tc.tile_pool(name="tmp3", bufs=2))
    psum = ctx.enter_context(tc.tile_pool(name="psum", bufs=2, space="PSUM"))
    dram = ctx.enter_context(tc.tile_pool(name="dram", bufs=1, space="DRAM"))

    # ---------------- persistent per-class tiles ----------------
    targT = [main.tile([P, N], F32, tag=f"targ{c}", name=f"targ{c}") for c in range(CL)]
    randT = [main.tile([P, N], F32, tag=f"rand{c}", name=f"rand{c}") for c in range(CL)]
    bceT = [main.tile([P, N], F32, tag=f"bce{c}", name=f"bce{c}") for c in range(CL)]
    gT = [main.tile([P, N], F32, tag=f"g{c}", name=f"g{c}") for c in range(CL)]
    gqT = [main.tile([P, N], mybir.dt.bfloat16, tag=f"gq{c}", name=f"gq{c}") for c in range(CL)]
    x2T = [main.tile([P, N], F32, tag=f"x2{c}", name=f"x2{c}") for c in range(CL)]

    # replicated per-class scalar tiles [P, CL] and accumulators
    accE = main.tile([P, CL], F32, tag="accE", name="accE")   # sum(1-2t) partials
    accL = main.tile([P, CL], F32, tag="accL", name="accL")   # sum(bce) partials
    scal_sb = main.tile([P, CL, 2], F32, tag="scal_sb", name="scal_sb")

    junkA = main.tile([P, N], F32, tag="junkA", name="junkA")
    junkD = main.tile([P, N], F32, tag="junkD", name="junkD")
    ones_n = main.tile([P, N], F32, tag="ones_n", name="ones_n")
    nc.vector.memset(ones_n, 1.0)
    ones_col = main.tile([P, 1], F32, tag="ones_col", name="ones_col")
    nc.vector.memset(ones_col, 1.0)
    n08_col = main.tile([P, 1], F32, tag="n08_col", name="n08_col")
    nc.vector.memset(n08_col, -0.8)
    ones5 = main.tile([CL, 1], F32, tag="ones5", name="ones5")
    nc.vector.memset(ones5, 1.0)
    I5 = main.tile([CL, CL], F32, tag="I5", name="I5")
    I5_d = nc.inline_tensor(np.eye(CL, dtype=np.float32), name="I5c")
    nc.sync.dma_start(out=I5, in_=I5_d.ap())

    def repl_reduce(acc, nm):
        """[P, CL] partials -> PSUM [P, CL] with per-class totals replicated."""
        p1 = psum.tile([CL, 1], F32, tag="p1", name=f"p1_{nm}")
        nc.tensor.matmul(out=p1, lhsT=acc, rhs=ones_col, start=True, stop=True)
        s1 = main.tile([CL, P], F32, tag="s1r", name=f"s1_{nm}")
        nc.vector.tensor_copy(s1, _bc(p1, P))
        p2 = psum.tile([P, CL], F32, tag="p2", name=f"p2_{nm}")
        nc.tensor.matmul(out=p2, lhsT=s1, rhs=I5, start=True, stop=True)
        return p2

    # ---------------- DMA in ----------------
    pred_ap = pred_d.ap()
    targ_ap = targ_d.ap()
    rand_ap = rand_d.ap()

    predT = []
    for c in range(CL):
        pt = tmp3.tile([P, N], F32, tag="t0", name="pred")
        nc.sync.dma_start(
            out=pt, in_=pred_ap[c : c + 1, :].rearrange("a (p n) -> (a p) n", p=P)
        )
        predT.append(pt)
        nc.sync.dma_start(
            out=targT[c],
            in_=targ_ap[c : c + 1, :].rearrange("a (p n) -> (a p) n", p=P),
        )
    sc_ap = scal_d.ap()
    sc_b = bass.AP(
        tensor=sc_ap.tensor, offset=sc_ap.offset, ap=[[0, P], sc_ap.ap[0], sc_ap.ap[1]]
    )
    nc.sync.dma_start(out=scal_sb, in_=sc_b)
    for c in range(CL):
        nc.sync.dma_start(
            out=randT[c],
            in_=rand_ap[c : c + 1, :].rearrange("a (p n) -> (a p) n", p=P),
        )

    # ---------------- phase E ----------------
    # x2 = pred*(1-2t); bce = 0.5*(x2+|x2|)+ln(1+exp(-|x2|)); g = sigmoid(x2)
    for c in range(CL):
        s2 = tmp3.tile([P, N], F32, tag="t1", name="s2")
        nc.scalar.activation(out=s2, in_=targT[c], func=AF.Identity,
                             scale=-2.0, bias=ones_col,
                             accum_out=accE[:, c : c + 1])
        nc.vector.tensor_mul(x2T[c], predT[c], s2)
        abits = tmp3.tile([P, N], F32, tag="t3", name="abits")
        nc.scalar.activation(out=abits, in_=x2T[c], func=AF.Abs)
        el = tmp3.tile([P, N], F32, tag="t4", name="el")
        nc.scalar.activation(out=el, in_=abits, func=AF.Exp, scale=-1.0)
        nc.scalar.activation(out=el, in_=el, func=AF.Ln, bias=1.0)
        r2v = tmp3.tile([P, N], F32, tag="t5", name="r2v")
        nc.gpsimd.tensor_tensor(out=r2v, in0=x2T[c], in1=abits, op=ALU.add)
        nc.vector.scalar_tensor_tensor(
            out=bceT[c], in0=r2v, scalar=0.5, in1=el,
            op0=ALU.mult, op1=ALU.add, accum_out=accL[:, c : c + 1],
        )
    for c in range(CL):
        nc.scalar.activation(out=gT[c], in_=x2T[c], func=AF.Sigmoid)

    # ---------------- per-class scalars (replicated [P, CL]) ----------------
    S2R = repl_reduce(accE, "S2R")
    possum = main.tile([P, CL], F32, tag="possum", name="possum")
    nc.vector.tensor_scalar(out=possum, in0=S2R, scalar1=-0.5, scalar2=BF / 2,
                            op0=ALU.mult, op1=ALU.add)
    balpos = main.tile([P, CL], F32, tag="balpos", name="balpos")
    nc.vector.tensor_scalar(out=balpos, in0=scal_sb[:, :, 0], scalar1=BF, scalar2=None,
                            op0=ALU.mult)
    posgt = main.tile([P, CL], F32, tag="posgt", name="posgt")
    nc.vector.tensor_tensor(out=posgt, in0=possum, in1=balpos, op=ALU.is_gt)
    posgt_m = main.tile([P, CL], U8, tag="posgt_m", name="posgt_m")
    nc.vector.tensor_tensor(out=posgt_m, in0=possum, in1=balpos, op=ALU.is_gt)
    kq = main.tile([P, CL], F32, tag="kq", name="kq")
    nc.vector.tensor_sub(kq, possum, balpos)
    nc.vector.tensor_scalar(out=kq.bitcast(mybir.dt.uint32), in0=kq.bitcast(mybir.dt.uint32),
                            scalar1=0x7FFFFFFF, scalar2=None, op0=ALU.bitwise_and)
    # notm = (target != majlab) = target*(1-2*posgt) + posgt
    u2 = main.tile([P, CL], F32, tag="u2", name="u2")
    nc.vector.tensor_scalar(out=u2, in0=posgt, scalar1=-2.0, scalar2=1.0,
                            op0=ALU.mult, op1=ALU.add)
    # balance = posgt ? balpos : B-balpos
    balance = main.tile([P, CL], F32, tag="balance", name="balance")
    nc.vector.tensor_scalar(out=balance, in0=balpos, scalar1=-1.0, scalar2=BF,
                            op0=ALU.mult, op1=ALU.add)
    nc.vector.copy_predicated(out=balance, mask=posgt_m, data=balpos)
    # majcnt = posgt ? possum : B-possum ; mincnt = B-majcnt
    majcnt = main.tile([P, CL], F32, tag="majcnt", name="majcnt")
    nc.vector.tensor_scalar(out=majcnt, in0=possum, scalar1=-1.0, scalar2=BF,
                            op0=ALU.mult, op1=ALU.add)
    nc.vector.copy_predicated(out=majcnt, mask=posgt_m, data=possum)
    mincnt = main.tile([P, CL], F32, tag="mincnt", name="mincnt")
    nc.vector.tensor_scalar(out=mincnt, in0=majcnt, scalar1=-1.0, scalar2=BF,
                            op0=ALU.mult, op1=ALU.add)
    rmaj = main.tile([P, CL], F32, tag="rmaj", name="rmaj")
    nc.vector.reciprocal(out=rmaj, in_=majcnt)
    rmin = main.tile([P, CL], F32, tag="rmin", name="rmin")
    nc.vector.reciprocal(out=rmin, in_=mincnt)
    alpha = main.tile([P, CL], F32, tag="alpha", name="alpha")
    nc.vector.tensor_mul(alpha, balance, rmaj)
    bminus = main.tile([P, CL], F32, tag="bminus", name="bminus")
    nc.vector.tensor_scalar(out=bminus, in0=balance, scalar1=-1.0, scalar2=BF,
                            op0=ALU.mult, op1=ALU.add)
    beta = main.tile([P, CL], F32, tag="beta", name="beta")
    nc.vector.tensor_mul(beta, bminus, rmin)

    # gmb = 2*notm + g in bf16: maj samples in (0,1), min samples in (2,3)
    for c in range(CL):
        notm = tmp3.tile([P, N], F32, tag="t2", name="notm")
        nc.vector.tensor_scalar(out=notm, in0=targT[c], scalar1=u2[:, c : c + 1],
                                scalar2=posgt[:, c : c + 1], op0=ALU.mult, op1=ALU.add)
        nc.vector.scalar_tensor_tensor(
            out=gqT[c], in0=notm, scalar=2.0, in1=gT[c],
            op0=ALU.mult, op1=ALU.add,
        )

    # ---------------- loss_c -> ln -> collective max ----------------
    LR = repl_reduce(accL, "LR")
    lnT = main.tile([P, CL], F32, tag="lnT", name="lnT")
    nc.scalar.activation(out=lnT, in_=LR, func=AF.Ln, bias=1.0)
    pair = main.tile([P, 2], F32, tag="pair", name="pair")
    nc.vector.tensor_reduce(out=pair[:, 0:1], in_=lnT, axis=mybir.AxisListType.X,
                            op=ALU.max)
    negln = main.tile([P, CL], F32, tag="negln", name="negln")
    nc.vector.tensor_scalar(out=negln, in0=lnT, scalar1=-1.0, scalar2=None, op0=ALU.mult)
    nc.vector.tensor_reduce(out=pair[:, 1:2], in_=negln, axis=mybir.AxisListType.X,
                            op=ALU.max)
    cc_in = dram.tile([1, 2], F32, name="cc_in")
    cc_out = dram.tile([1, 2], F32, name="cc_out")
    nc.sync.dma_start(out=cc_in, in_=pair[0:1, :])
    nc.gpsimd.collective_compute(
        "AllReduce", ALU.max,
        replica_groups=[list(range(NCORES))],
        ins=[cc_in[:].opt()], outs=[cc_out[:].opt()],
    )
    mx = main.tile([P, 2], F32, tag="mx", name="mx")
    cc_out_ap = cc_out[:]
    mx_b = bass.AP(tensor=cc_out_ap.tensor, offset=cc_out_ap.offset,
                   ap=[[0, P], cc_out_ap.ap[1]])
    nc.sync.dma_start(out=mx, in_=mx_b)

    # drate = sigmoid(5 - 10*(ln - lnmin)/(lnmax - lnmin))
    den = main.tile([P, 1], F32, tag="den", name="den")
    nc.vector.tensor_add(den, mx[:, 0:1], mx[:, 1:2])
    rden = main.tile([P, 1], F32, tag="rden", name="rden")
    nc.vector.reciprocal(out=rden, in_=den)
    num = main.tile([P, CL], F32, tag="num", name="num")
    nc.vector.tensor_add(num, lnT, _bc(mx[:, 1:2], CL))
    nc.vector.tensor_mul(num, num, _bc(rden, CL))
    norm = main.tile([P, CL], F32, tag="norm", name="norm")
    nc.vector.tensor_scalar(out=norm, in0=num, scalar1=-10.0, scalar2=5.0,
                            op0=ALU.mult, op1=ALU.add)
    drate = main.tile([P, CL], F32, tag="drate", name="drate")
    nc.scalar.activation(out=drate, in_=norm, func=AF.Sigmoid)
    ndrate = main.tile([P, CL], F32, tag="ndrate", name="ndrate")
    nc.vector.tensor_scalar(out=ndrate, in0=drate, scalar1=-1.0, scalar2=None,
                            op0=ALU.mult)
    hardf = main.tile([P, CL], U8, tag="hardf", name="hardf")
    nc.vector.tensor_tensor(out=hardf, in0=scal_sb[:, :, 1], in1=drate, op=ALU.is_gt)

    # folded weight scalars: w = target*A2 + B2, then -0.5 for the sign trick
    am1 = main.tile([P, CL], F32, tag="am1", name="am1")
    nc.vector.tensor_scalar(out=am1, in0=alpha, scalar1=-1.0, scalar2=None, op0=ALU.add)
    Aw = main.tile([P, CL], F32, tag="Aw", name="Aw")
    nc.vector.tensor_scalar(out=Aw, in0=beta, scalar1=-1.0, scalar2=1.0,
                            op0=ALU.mult, op1=ALU.add)
    nc.vector.copy_predicated(out=Aw, mask=hardf, data=am1)
    ones = main.tile([P, CL], F32, tag="ones", name="ones")
    nc.vector.memset(ones, 1.0)
    Bw = main.tile([P, CL], F32, tag="Bw", name="Bw")
    nc.vector.tensor_copy(Bw, beta)
    nc.vector.copy_predicated(out=Bw, mask=hardf, data=ones)
    A2 = main.tile([P, CL], F32, tag="A2", name="A2")
    nc.vector.tensor_mul(A2, u2, Aw)
    B2s = main.tile([P, CL], F32, tag="B2s", name="B2s")
    nc.vector.tensor_mul(B2s, posgt, Aw)
    nc.vector.tensor_add(B2s, B2s, Bw)
    nc.vector.tensor_scalar(out=B2s, in0=B2s, scalar1=-0.5, scalar2=None, op0=ALU.add)

    # ---------------- bisection for per-class k-th smallest ----------------
    loE = main.tile([P, CL], F32, tag="loE", name="loE")
    nc.vector.memset(loE, 0.0)
    thr = main.tile([P, CL], F32, tag="thr", name="thr")
    # ACT classes (0..2): stat = sum sign(te-gmb) = 2cnt-B -> thr = 2k-B;
    # DVE classes (3..4): stat = cnt -> thr = k. flag_hi = stat > thr.
    nc.vector.tensor_copy(thr, kq)
    nc.vector.tensor_scalar(out=thr[:, 0:2], in0=kq[:, 0:2], scalar1=2.0, scalar2=-BF,
                            op0=ALU.mult, op1=ALU.add)
    bacc_t = main.tile([P, CL], F32, tag="bacc", name="bacc")
    te = main.tile([P, CL], F32, tag="te", name="te")
    flo_f = main.tile([P, CL], F32, tag="flo_f", name="flo_f")

    for it in range(BIS_ITERS):
        w_half = 2.0 ** -(it + 1)
        nc.vector.tensor_scalar(out=te, in0=loE, scalar1=w_half, scalar2=None,
                                op0=ALU.add)
        for c in range(2):
            nc.scalar.activation(out=junkA, in_=gqT[c], func=AF.Sign,
                                 scale=-1.0, bias=te[:, c : c + 1],
                                 accum_out=bacc_t[:, c : c + 1])
        for c in range(2, CL):
            nc.vector.tensor_scalar(out=junkD, in0=gqT[c], scalar1=te[:, c : c + 1],
                                    scalar2=None, op0=ALU.is_lt, op1=ALU.add,
                                    accum_out=bacc_t[:, c : c + 1])
        bRp = repl_reduce(bacc_t, f"bR{it}")
        nc.vector.tensor_tensor(out=flo_f, in0=bRp, in1=thr, op=ALU.is_le)
        nc.vector.scalar_tensor_tensor(out=loE, in0=flo_f, scalar=w_half, in1=loE,
                                       op0=ALU.mult, op1=ALU.add)

    # final threshold (already offset); hard classes get sentinel -9
    tadj = main.tile([P, CL], F32, tag="tadj", name="tadj")
    nc.vector.tensor_scalar(out=tadj, in0=loE, scalar1=2.0 ** -(BIS_ITERS + 1),
                            scalar2=None, op0=ALU.add)
    tstar = main.tile([P, CL], F32, tag="tstar", name="tstar")
    nc.vector.tensor_copy(tstar, tadj)
    neg9 = main.tile([P, CL], F32, tag="neg9", name="neg9")
    nc.vector.memset(neg9, -9.0)
    nc.vector.copy_predicated(out=tadj, mask=hardf, data=neg9)
    ntadj = main.tile([P, CL], F32, tag="ntadj", name="ntadj")
    nc.vector.tensor_scalar(out=ntadj, in0=tadj, scalar1=-1.0, scalar2=None,
                            op0=ALU.mult)

    # ---------------- final weighted sum ----------------
    # m1 = bce*(1 - di*ix) with di = (rand>drate), ix = (g>=0.8);
    # total_c = sum(m1*w1) + 0.5*sum(m1*sign(gq-tadj))
    f1acc = main.tile([P, CL], F32, tag="f1acc", name="f1acc")
    f2acc = main.tile([P, CL], F32, tag="f2acc", name="f2acc")
    m1T = []
    for c in range(CL):
        sF = tmp3.tile([P, N], F32, tag="t2", name="sF")
        nc.scalar.activation(out=sF, in_=randT[c], func=AF.Sign,
                             bias=ndrate[:, c : c + 1])
        six = tmp3.tile([P, N], F32, tag="t3", name="six")
        nc.scalar.activation(out=six, in_=gT[c], func=AF.Sign, bias=n08_col)
        ssum = tmp3.tile([P, N], F32, tag="t4", name="ssum")
        nc.vector.tensor_add(ssum, sF, six)
        nfi = tmp3.tile([P, N], F32, tag="t5", name="nfi")
        nc.vector.tensor_scalar(out=nfi, in0=ssum, scalar1=1.9, scalar2=None,
                                op0=ALU.is_lt)
        m1 = tmp3.tile([P, N], F32, tag="t6", name="m1", bufs=6)
        nc.vector.tensor_mul(m1, bceT[c], nfi)
        m1T.append(m1)
        w1 = tmp3.tile([P, N], F32, tag="t1", name="w1")
        nc.scalar.activation(out=w1, in_=targT[c], func=AF.Identity,
                             scale=A2[:, c : c + 1], bias=B2s[:, c : c + 1])
        nc.vector.scalar_tensor_tensor(out=junkD, in0=m1, scalar=1.0, in1=w1,
                                       op0=ALU.mult, op1=ALU.mult,
                                       accum_out=f1acc[:, c : c + 1])
    for c in range(CL):
        sg = tmp3.tile([P, N], F32, tag="t0", name="sg")
        nc.scalar.activation(out=sg, in_=gqT[c], func=AF.Sign,
                             bias=ntadj[:, c : c + 1])
        nc.vector.scalar_tensor_tensor(out=junkD, in0=m1T[c], scalar=1.0, in1=sg,
                                       op0=ALU.mult, op1=ALU.mult,
                                       accum_out=f2acc[:, c : c + 1])

    diff = main.tile([P, CL], F32, tag="diff", name="diff")
    nc.vector.scalar_tensor_tensor(out=diff, in0=f2acc, scalar=0.5, in1=f1acc,
                                   op0=ALU.mult, op1=ALU.add)
    # total = sum over partitions and classes via two tiny matmuls
    pT = psum.tile([CL, 1], F32, tag="p1", name="pT")
    nc.tensor.matmul(out=pT, lhsT=diff, rhs=ones_col, start=True, stop=True)
    s5 = main.tile([CL, 1], F32, tag="s5", name="s5")
    nc.vector.tensor_copy(s5, pT)
    pS = psum.tile([1, 1], F32, tag="p2", name="pS")
    nc.tensor.matmul(out=pS, lhsT=s5, rhs=ones5, start=True, stop=True)

    outt = main.tile([1, NOUT], F32, tag="outt", name="outt")
    nc.vector.memset(outt, 0.0)
    nc.vector.tensor_copy(outt[0:1, 0:1], pS)
    # debug slots
    nc.vector.tensor_copy(outt[0:1, 1 : 1 + CL], tstar[0:1, :])
    nc.vector.tensor_copy(outt[0:1, 6 : 6 + CL], kq[0:1, :])
    nc.vector.tensor_copy(outt[0:1, 11 : 11 + CL], possum[0:1, :])
    nc.vector.tensor_copy(outt[0:1, 21 : 21 + CL], drate[0:1, :])
    nc.vector.tensor_copy(outt[0:1, 26 : 26 + CL], hardf[0:1, :])
    nc.sync.dma_start(out=out_d[:, :], in_=outt)
    ctx.close()


def build_nc():
    nc = bacc.Bacc("TRN2", target_bir_lowering=False, debug=False,
                   num_devices=NCORES)
    pred_d = nc.dram_tensor("pred", [CL, B], F32, kind="ExternalInput")
    targ_d = nc.dram_tensor("target", [CL, B], F32, kind="ExternalInput")
    rand_d = nc.dram_tensor("rand", [CL, B], F32, kind="ExternalInput")
    scal_d = nc.dram_tensor("scal", [CL, 2], F32, kind="ExternalInput")
    out_d = nc.dram_tensor("out", [1, NOUT], F32, kind="ExternalOutput")
    with tile.TileContext(nc) as tc:
        _body(tc, pred_d, targ_d, rand_d, scal_d, out_d)
    nc.compile()
    return nc


def _shard(pred, target, rand_mat, hard_rand, pos_prop):
    in_maps = []
    for i in range(NCORES):
        sl = slice(CL * i, CL * (i + 1))
        in_maps.append({
            "pred": np.ascontiguousarray(pred[:, sl].T),
            "target": np.ascontiguousarray(target[:, sl].T),
            "rand": np.ascontiguousarray(rand_mat[:, sl].T),
            "scal": np.ascontiguousarray(
                np.stack([pos_prop[sl], hard_rand[0, sl]], axis=1)
            ).astype(np.float32),
        })
    return in_maps


def _gather(res):
    total = sum(float(r["out"][0, 0]) for r in res.results)
    return np.array(total / (B * C), dtype=np.float32)


def kernel(pred, target, rand_mat, hard_rand, pos_prop):
    if "nc" not in _CACHED:
        _CACHED["nc"] = build_nc()
    nc = _CACHED["nc"]
    in_maps = _shard(pred, target, rand_mat, hard_rand, pos_prop)
    # the first exec after a fresh NEFF load occasionally dies with a
    # transient NRT_EXEC_UNIT_UNRECOVERABLE; one retry recovers it
    try:
        res = run_bass_kernel_spmd(nc, in_maps, core_ids=list(range(NCORES)))
    except Exception:
        import time as _time
        _time.sleep(2.0)
        res = run_bass_kernel_spmd(nc, in_maps, core_ids=list(range(NCORES)))
    return _gather(res)


def _install_ntff_shim():
    """The agent image's antenv lacks axon_hooks; shim it from trn_boot."""
    import sys, types
    try:
        from antenv import axon_hooks  # noqa: F401
        return
    except ImportError:
        pass
    import antenv
    from trn_agent_boot.trn_boot import _ntff_profile_via_ctypes
    hook = _ntff_profile_via_ctypes("/opt/axon/libaxon_pjrt.so")
    mod = types.ModuleType("antenv.axon_hooks")
    mod.get_axon_ntff_profile_hook = lambda: hook
    mod.set_axon_ntff_profile_hook = lambda h: None
    sys.modules["antenv.axon_hooks"] = mod
    antenv.axon_hooks = mod
    # artifact upload has no cloud access here; keep traces local
    import concourse.bass_utils as bu
    bu.upload_artifacts = lambda tmpdir: tmpdir


def kernel_profiled(pred, target, rand_mat, hard_rand, pos_prop, **trace_kwargs):
    """Same as kernel() but with NTFF tracing; returns (out, BassKernelResults)."""
    _install_ntff_shim()
    if "nc" not in _CACHED:
        _CACHED["nc"] = build_nc()
    nc = _CACHED["nc"]
    in_maps = _shard(pred, target, rand_mat, hard_rand, pos_prop)
    res = run_bass_kernel_spmd(nc, in_maps, core_ids=list(range(NCORES)),
                               trace=True, **trace_kwargs)
    return _gather(res), res


# revision 32
# speedup vs baseline: 1.0318x; 1.0318x over previous
"""Trainium2 Bass kernel for PosNegBalanceLoss.

Contract: kernel(**inputs) takes FULL unsharded inputs (pred/target/rand_mat
[131072,40] f32, hard_rand [1,40], pos_prop [40]) and returns the FULL scalar
output, distributing across 8 NeuronCores internally.

Sharding: by class columns (40 classes -> 5 per core). Each core gets
contiguous [5, 131072] slices (host-side transpose), so every per-class
reduction including the rank/argsort step is core-local. The only cross-core
step is an 8-byte AllReduce(max) of (ln_max, -ln_min) over per-class losses.

Math: with x2 = pred*(1-2*target), bce = logaddexp(0,pred) - pred*target
== softplus(x2), and g = |sigmoid(pred)-target| == sigmoid(x2), so the whole
elementwise phase is two DVE ops + two ACT table ops per class.

The rank step (drop the dropout_num smallest-g majority samples per class) is
a per-class threshold search: lockstep bisection with exact counts of
(g_masked < t) via ACT Sign+accum / DVE is_ge+accum; cross-partition count
totals are replicated with a PE double-matmul (acc @ ones -> [5,1];
broadcast-copy -> [5,128]; @ I5 -> [128,5]). Per-class scalars stay
replicated across partitions the whole time, so no partition broadcasts are
needed. A fixed number of bisection rounds leaves ~2e-5 relative error on the
final mean (only elements inside the final bisection interval can be
mis-dropped, each contributing ~2e-7).
"""

import numpy as np
from contextlib import ExitStack

import concourse.bass as bass
import concourse.tile as tile
from concourse import bacc, mybir
from concourse.bass_utils import run_bass_kernel_spmd

F32 = mybir.dt.float32
U8 = mybir.dt.uint8
ALU = mybir.AluOpType
AF = mybir.ActivationFunctionType

B = 131072            # batch (rows)
C = 40                # classes (cols)
NCORES = 8
CL = C // NCORES      # classes per core = 5
P = 128               # sbuf partitions
N = B // P            # 1024 elems per partition per class
BF = float(B)
BIS_ITERS = 9         # bisection rounds
NOUT = 32             # debug-friendly output vector per core

_CACHED = {}


def _bc(ap, n):
    """Broadcast a [p,1] AP along the free dim to [p,n] (stride-0)."""
    return bass.AP(tensor=ap.tensor, offset=ap.offset, ap=[ap.ap[0], [0, n]])


def _body(tc: tile.TileContext, pred_d, targ_d, rand_d, scal_d, out_d):
    nc = tc.nc
    ctx = ExitStack()
    main = ctx.enter_context(tc.tile_pool(name="main", bufs=1))
    tmp3 = ctx.enter_context(tc.tile_pool(name="tmp3", bufs=2))
    psum = ctx.enter_context(tc.tile_pool(name="psum", bufs=2, space="PSUM"))
    dram = ctx.enter_context(tc.tile_pool(name="dram", bufs=1, space="DRAM"))

    # ---------------- persistent per-class tiles ----------------
    targT = [main.tile([P, N], F32, tag=f"targ{c}", name=f"targ{c}") for c in range(CL)]
    randT = [main.tile([P, N], F32, tag=f"rand{c}", name=f"rand{c}") for c in range(CL)]
    bceT = [main.tile([P, N], F32, tag=f"bce{c}", name=f"bce{c}") for c in range(CL)]
    gT = [main.tile([P, N], F32, tag=f"g{c}", name=f"g{c}") for c in range(CL)]
    gqT = [main.tile([P, N], mybir.dt.bfloat16, tag=f"gq{c}", name=f"gq{c}") for c in range(CL)]
    x2T = [main.tile([P, N], F32, tag=f"x2{c}", name=f"x2{c}") for c in range(CL)]

    # replicated per-class scalar tiles [P, CL] and accumulators
    accE = main.tile([P, CL], F32, tag="accE", name="accE")   # sum(1-2t) partials
    accL = main.tile([P, CL], F32, tag="accL", name="accL")   # sum(bce) partials
    scal_sb = main.tile([P, CL, 2], F32, tag="scal_sb", name="scal_sb")

    junkA = main.tile([P, N], F32, tag="junkA", name="junkA")
    junkD = main.tile([P, N], F32, tag="junkD", name="junkD")
    ones_n = main.tile([P, N], F32, tag="ones_n", name="ones_n")
    nc.vector.memset(ones_n, 1.0)
    ones_col = main.tile([P, 1], F32, tag="ones_col", name="ones_col")
    nc.vector.memset(ones_col, 1.0)
    n08_col = main.tile([P, 1], F32, tag="n08_col", name="n08_col")
    nc.vector.memset(n08_col, -0.8)
    ones5 = main.tile([CL, 1], F32, tag="ones5", name="ones5")
    nc.vector.memset(ones5, 1.0)
    I5 = main.tile([CL, CL], F32, tag="I5", name="I5")
    I5_d = nc.inline_tensor(np.eye(CL, dtype=np.float32), name="I5c")
    nc.sync.dma_start(out=I5, in_=I5_d.ap())

    def repl_reduce(acc, nm):
        """[P, CL] partials -> PSUM [P, CL] with per-class totals replicated."""
        p1 = psum.tile([CL, 1], F32, tag="p1", name=f"p1_{nm}")
        nc.tensor.matmul(out=p1, lhsT=acc, rhs=ones_col, start=True, stop=True)
        s1 = main.tile([CL, P], F32, tag="s1r", name=f"s1_{nm}")
        nc.vector.tensor_copy(s1, _bc(p1, P))
        p2 = psum.tile([P, CL], F32, tag="p2", name=f"p2_{nm}")
        nc.tensor.matmul(out=p2, lhsT=s1, rhs=I5, start=True, stop=True)
        return p2

    # ---------------- DMA in ----------------
    pred_ap = pred_d.ap()
    targ_ap = targ_d.ap()
    rand_ap = rand_d.ap()

    predT = []
    for c in range(CL):
        pt = tmp3.tile([P, N], F32, tag="t0", name="pred")
        nc.sync.dma_start(
            out=pt, in_=pred_ap[c : c + 1, :].rearrange("a (p n) -> (a p) n", p=P)
        )
        predT.append(pt)
        nc.sync.dma_start(
            out=targT[c],
            in_=targ_ap[c : c + 1, :].rearrange("a (p n) -> (a p) n", p=P),
        )
    sc_ap = scal_d.ap()
    sc_b = bass.AP(
        tensor=sc_ap.tensor, offset=sc_ap.offset, ap=[[0, P], sc_ap.ap[0], sc_ap.ap[1]]
    )
    nc.sync.dma_start(out=scal_sb, in_=sc_b)
    for c in range(CL):
        nc.sync.dma_start(
            out=randT[c],
            in_=rand_ap[c : c + 1, :].rearrange("a (p n) -> (a p) n", p=P),
        )

    # ---------------- phase E ----------------
    # x2 = pred*(1-2t); bce = 0.5*(x2+|x2|)+ln(1+exp(-|x2|)); g = sigmoid(x2)
    for c in range(CL):
        s2 = tmp3.tile([P, N], F32, tag="t1", name="s2")
        nc.vector.scalar_tensor_tensor(
            out=s2, in0=targT[c], scalar=-2.0, in1=ones_n,
            op0=ALU.mult, op1=ALU.add, accum_out=accE[:, c : c + 1],
        )
        nc.vector.tensor_mul(x2T[c], predT[c], s2)
        abits = tmp3.tile([P, N], F32, tag="t3", name="abits")
        nc.vector.tensor_scalar(out=abits.bitcast(mybir.dt.uint32),
                                in0=x2T[c].bitcast(mybir.dt.uint32),
                                scalar1=0x7FFFFFFF, scalar2=None, op0=ALU.bitwise_and)
        el = tmp3.tile([P, N], F32, tag="t4", name="el")
        nc.scalar.activation(out=el, in_=abits, func=AF.Exp, scale=-1.0)
        nc.scalar.activation(out=el, in_=el, func=AF.Ln, bias=1.0)
        r2v = tmp3.tile([P, N], F32, tag="t5", name="r2v")
        nc.gpsimd.tensor_tensor(out=r2v, in0=x2T[c], in1=abits, op=ALU.add)
        nc.vector.scalar_tensor_tensor(
            out=bceT[c], in0=r2v, scalar=0.5, in1=el,
            op0=ALU.mult, op1=ALU.add, accum_out=accL[:, c : c + 1],
        )
    for c in range(CL):
        nc.scalar.activation(out=gT[c], in_=x2T[c], func=AF.Sigmoid)

    # ---------------- per-class scalars (replicated [P, CL]) ----------------
    S2R = repl_reduce(accE, "S2R")
    possum = main.tile([P, CL], F32, tag="possum", name="possum")
    nc.vector.tensor_scalar(out=possum, in0=S2R, scalar1=-0.5, scalar2=BF / 2,
                            op0=ALU.mult, op1=ALU.add)
    balpos = main.tile([P, CL], F32, tag="balpos", name="balpos")
    nc.vector.tensor_scalar(out=balpos, in0=scal_sb[:, :, 0], scalar1=BF, scalar2=None,
                            op0=ALU.mult)
    posgt = main.tile([P, CL], F32, tag="posgt", name="posgt")
    nc.vector.tensor_tensor(out=posgt, in0=possum, in1=balpos, op=ALU.is_gt)
    posgt_m = main.tile([P, CL], U8, tag="posgt_m", name="posgt_m")
    nc.vector.tensor_tensor(out=posgt_m, in0=possum, in1=balpos, op=ALU.is_gt)
    kq = main.tile([P, CL], F32, tag="kq", name="kq")
    nc.vector.tensor_sub(kq, possum, balpos)
    nc.vector.tensor_scalar(out=kq.bitcast(mybir.dt.uint32), in0=kq.bitcast(mybir.dt.uint32),
                            scalar1=0x7FFFFFFF, scalar2=None, op0=ALU.bitwise_and)
    # notm = (target != majlab) = target*(1-2*posgt) + posgt
    u2 = main.tile([P, CL], F32, tag="u2", name="u2")
    nc.vector.tensor_scalar(out=u2, in0=posgt, scalar1=-2.0, scalar2=1.0,
                            op0=ALU.mult, op1=ALU.add)
    # balance = posgt ? balpos : B-balpos
    balance = main.tile([P, CL], F32, tag="balance", name="balance")
    nc.vector.tensor_scalar(out=balance, in0=balpos, scalar1=-1.0, scalar2=BF,
                            op0=ALU.mult, op1=ALU.add)
    nc.vector.copy_predicated(out=balance, mask=posgt_m, data=balpos)
    # majcnt = posgt ? possum : B-possum ; mincnt = B-majcnt
    majcnt = main.tile([P, CL], F32, tag="majcnt", name="majcnt")
    nc.vector.tensor_scalar(out=majcnt, in0=possum, scalar1=-1.0, scalar2=BF,
                            op0=ALU.mult, op1=ALU.add)
    nc.vector.copy_predicated(out=majcnt, mask=posgt_m, data=possum)
    mincnt = main.tile([P, CL], F32, tag="mincnt", name="mincnt")
    nc.vector.tensor_scalar(out=mincnt, in0=majcnt, scalar1=-1.0, scalar2=BF,
                            op0=ALU.mult, op1=ALU.add)
    rmaj = main.tile([P, CL], F32, tag="rmaj", name="rmaj")
    nc.vector.reciprocal(out=rmaj, in_=majcnt)
    rmin = main.tile([P, CL], F32, tag="rmin", name="rmin")
    nc.vector.reciprocal(out=rmin, in_=mincnt)
    alpha = main.tile([P, CL], F32, tag="alpha", name="alpha")
    nc.vector.tensor_mul(alpha, balance, rmaj)
    bminus = main.tile([P, CL], F32, tag="bminus", name="bminus")
    nc.vector.tensor_scalar(out=bminus, in0=balance, scalar1=-1.0, scalar2=BF,
                            op0=ALU.mult, op1=ALU.add)
    beta = main.tile([P, CL], F32, tag="beta", name="beta")
    nc.vector.tensor_mul(beta, bminus, rmin)

    # gmb = 2*notm + g in bf16: maj samples in (0,1), min samples in (2,3)
    for c in range(CL):
        notm = tmp3.tile([P, N], F32, tag="t2", name="notm")
        nc.vector.tensor_scalar(out=notm, in0=targT[c], scalar1=u2[:, c : c + 1],
                                scalar2=posgt[:, c : c + 1], op0=ALU.mult, op1=ALU.add)
        nc.vector.scalar_tensor_tensor(
            out=gqT[c], in0=notm, scalar=2.0, in1=gT[c],
            op0=ALU.mult, op1=ALU.add,
        )

    # ---------------- loss_c -> ln -> collective max ----------------
    LR = repl_reduce(accL, "LR")
    lnT = main.tile([P, CL], F32, tag="lnT", name="lnT")
    nc.scalar.activation(out=lnT, in_=LR, func=AF.Ln, bias=1.0)
    pair = main.tile([P, 2], F32, tag="pair", name="pair")
    nc.vector.tensor_reduce(out=pair[:, 0:1], in_=lnT, axis=mybir.AxisListType.X,
                            op=ALU.max)
    negln = main.tile([P, CL], F32, tag="negln", name="negln")
    nc.vector.tensor_scalar(out=negln, in0=lnT, scalar1=-1.0, scalar2=None, op0=ALU.mult)
    nc.vector.tensor_reduce(out=pair[:, 1:2], in_=negln, axis=mybir.AxisListType.X,
                            op=ALU.max)
    cc_in = dram.tile([1, 2], F32, name="cc_in")
    cc_out = dram.tile([1, 2], F32, name="cc_out")
    nc.sync.dma_start(out=cc_in, in_=pair[0:1, :])
    nc.gpsimd.collective_compute(
        "AllReduce", ALU.max,
        replica_groups=[list(range(NCORES))],
        ins=[cc_in[:].opt()], outs=[cc_out[:].opt()],
    )
    mx = main.tile([P, 2], F32, tag="mx", name="mx")
    cc_out_ap = cc_out[:]
    mx_b = bass.AP(tensor=cc_out_ap.tensor, offset=cc_out_ap.offset,
                   ap=[[0, P], cc_out_ap.ap[1]])
    nc.sync.dma_start(out=mx, in_=mx_b)

    # drate = sigmoid(5 - 10*(ln - lnmin)/(lnmax - lnmin))
    den = main.tile([P, 1], F32, tag="den", name="den")
    nc.vector.tensor_add(den, mx[:, 0:1], mx[:, 1:2])
    rden = main.tile([P, 1], F32, tag="rden", name="rden")
    nc.vector.reciprocal(out=rden, in_=den)
    num = main.tile([P, CL], F32, tag="num", name="num")
    nc.vector.tensor_add(num, lnT, _bc(mx[:, 1:2], CL))
    nc.vector.tensor_mul(num, num, _bc(rden, CL))
    norm = main.tile([P, CL], F32, tag="norm", name="norm")
    nc.vector.tensor_scalar(out=norm, in0=num, scalar1=-10.0, scalar2=5.0,
                            op0=ALU.mult, op1=ALU.add)
    drate = main.tile([P, CL], F32, tag="drate", name="drate")
    nc.scalar.activation(out=drate, in_=norm, func=AF.Sigmoid)
    ndrate = main.tile([P, CL], F32, tag="ndrate", name="ndrate")
    nc.vector.tensor_scalar(out=ndrate, in0=drate, scalar1=-1.0, scalar2=None,
                            op0=ALU.mult)
    hardf = main.tile([P, CL], U8, tag="hardf", name="hardf")
    nc.vector.tensor_tensor(out=hardf, in0=scal_sb[:, :, 1], in1=drate, op=ALU.is_gt)

    # folded weight scalars: w = target*A2 + B2, then -0.5 for the sign trick
    am1 = main.tile([P, CL], F32, tag="am1", name="am1")
    nc.vector.tensor_scalar(out=am1, in0=alpha, scalar1=-1.0, scalar2=None, op0=ALU.add)
    Aw = main.tile([P, CL], F32, tag="Aw", name="Aw")
    nc.vector.tensor_scalar(out=Aw, in0=beta, scalar1=-1.0, scalar2=1.0,
                            op0=ALU.mult, op1=ALU.add)
    nc.vector.copy_predicated(out=Aw, mask=hardf, data=am1)
    ones = main.tile([P, CL], F32, tag="ones", name="ones")
    nc.vector.memset(ones, 1.0)
    Bw = main.tile([P, CL], F32, tag="Bw", name="Bw")
    nc.vector.tensor_copy(Bw, beta)
    nc.vector.copy_predicated(out=Bw, mask=hardf, data=ones)
    A2 = main.tile([P, CL], F32, tag="A2", name="A2")
    nc.vector.tensor_mul(A2, u2, Aw)
    B2s = main.tile([P, CL], F32, tag="B2s", name="B2s")
    nc.vector.tensor_mul(B2s, posgt, Aw)
    nc.vector.tensor_add(B2s, B2s, Bw)
    nc.vector.tensor_scalar(out=B2s, in0=B2s, scalar1=-0.5, scalar2=None, op0=ALU.add)

    # ---------------- bisection for per-class k-th smallest ----------------
    loE = main.tile([P, CL], F32, tag="loE", name="loE")
    nc.vector.memset(loE, 0.0)
    thr = main.tile([P, CL], F32, tag="thr", name="thr")
    # ACT classes (0..2): stat = sum sign(te-gmb) = 2cnt-B -> thr = 2k-B;
    # DVE classes (3..4): stat = cnt -> thr = k. flag_hi = stat > thr.
    nc.vector.tensor_copy(thr, kq)
    nc.vector.tensor_scalar(out=thr[:, 0:2], in0=kq[:, 0:2], scalar1=2.0, scalar2=-BF,
                            op0=ALU.mult, op1=ALU.add)
    bacc_t = main.tile([P, CL], F32, tag="bacc", name="bacc")
    te = main.tile([P, CL], F32, tag="te", name="te")
    flo_f = main.tile([P, CL], F32, tag="flo_f", name="flo_f")

    for it in range(BIS_ITERS):
        w_half = 2.0 ** -(it + 1)
        nc.vector.tensor_scalar(out=te, in0=loE, scalar1=w_half, scalar2=None,
                                op0=ALU.add)
        for c in range(2):
            nc.scalar.activation(out=junkA, in_=gqT[c], func=AF.Sign,
                                 scale=-1.0, bias=te[:, c : c + 1],
                                 accum_out=bacc_t[:, c : c + 1])
        for c in range(2, CL):
            nc.vector.tensor_scalar(out=junkD, in0=gqT[c], scalar1=te[:, c : c + 1],
                                    scalar2=None, op0=ALU.is_lt, op1=ALU.add,
                                    accum_out=bacc_t[:, c : c + 1])
        bRp = repl_reduce(bacc_t, f"bR{it}")
        nc.vector.tensor_tensor(out=flo_f, in0=bRp, in1=thr, op=ALU.is_le)
        nc.vector.scalar_tensor_tensor(out=loE, in0=flo_f, scalar=w_half, in1=loE,
                                       op0=ALU.mult, op1=ALU.add)

    # final threshold (already offset); hard classes get sentinel -9
    tadj = main.tile([P, CL], F32, tag="tadj", name="tadj")
    nc.vector.tensor_scalar(out=tadj, in0=loE, scalar1=2.0 ** -(BIS_ITERS + 1),
                            scalar2=None, op0=ALU.add)
    tstar = main.tile([P, CL], F32, tag="tstar", name="tstar")
    nc.vector.tensor_copy(tstar, tadj)
    neg9 = main.tile([P, CL], F32, tag="neg9", name="neg9")
    nc.vector.memset(neg9, -9.0)
    nc.vector.copy_predicated(out=tadj, mask=hardf, data=neg9)
    ntadj = main.tile([P, CL], F32, tag="ntadj", name="ntadj")
    nc.vector.tensor_scalar(out=ntadj, in0=tadj, scalar1=-1.0, scalar2=None,
                            op0=ALU.mult)

    # ---------------- final weighted sum ----------------
    # m1 = bce*(1 - di*ix) with di = (rand>drate), ix = (g>=0.8);
    # total_c = sum(m1*w1) + 0.5*sum(m1*sign(gq-tadj))
    f1acc = main.tile([P, CL], F32, tag="f1acc", name="f1acc")
    f2acc = main.tile([P, CL], F32, tag="f2acc", name="f2acc")
    m1T = []
    for c in range(CL):
        sF = tmp3.tile([P, N], F32, tag="t2", name="sF")
        nc.scalar.activation(out=sF, in_=randT[c], func=AF.Sign,
                             bias=ndrate[:, c : c + 1])
        six = tmp3.tile([P, N], F32, tag="t3", name="six")
        nc.scalar.activation(out=six, in_=gT[c], func=AF.Sign, bias=n08_col)
        ssum = tmp3.tile([P, N], F32, tag="t4", name="ssum")
        nc.vector.tensor_add(ssum, sF, six)
        nfi = tmp3.tile([P, N], F32, tag="t5", name="nfi")
        nc.vector.tensor_scalar(out=nfi, in0=ssum, scalar1=1.9, scalar2=None,
                                op0=ALU.is_lt)
        m1 = tmp3.tile([P, N], F32, tag="t6", name="m1", bufs=6)
        nc.vector.tensor_mul(m1, bceT[c], nfi)
        m1T.append(m1)
        w1 = tmp3.tile([P, N], F32, tag="t1", name="w1")
        nc.scalar.activation(out=w1, in_=targT[c], func=AF.Identity,
                             scale=A2[:, c : c + 1], bias=B2s[:, c : c + 1])
        nc.vector.scalar_tensor_tensor(out=junkD, in0=m1, scalar=1.0, in1=w1,
                                       op0=ALU.mult, op1=ALU.mult,
                                       accum_out=f1acc[:, c : c + 1])
    for c in range(CL):
        sg = tmp3.tile([P, N], F32, tag="t0", name="sg")
        nc.scalar.activation(out=sg, in_=gqT[c], func=AF.Sign,
                             bias=ntadj[:, c : c + 1])
        nc.vector.scalar_tensor_tensor(out=junkD, in0=m1T[c], scalar=1.0, in1=sg,
                                       op0=ALU.mult, op1=ALU.mult,
                                       accum_out=f2acc[:, c : c + 1])

    diff = main.tile([P, CL], F32, tag="diff", name="diff")
    nc.vector.scalar_tensor_tensor(out=diff, in0=f2acc, scalar=0.5, in1=f1acc,
                                   op0=ALU.mult, op1=ALU.add)
    # total = sum over partitions and classes via two tiny matmuls
    pT = psum.tile([CL, 1], F32, tag="p1", name="pT")
    nc.tensor.matmul(out=pT, lhsT=diff, rhs=ones_col, start=True, stop=True)
    s5 = main.tile([CL, 1], F32, tag="s5", name="s5")
    nc.vector.tensor_copy(s5, pT)
    pS = psum.tile([1, 1], F32, tag="p2", name="pS")
    nc.tensor.matmul(out=pS, lhsT=s5, rhs=ones5, start=True, stop=True)

    outt = main.tile([1, NOUT], F32, tag="outt", name="outt")
    nc.vector.memset(outt, 0.0)
    nc.vector.tensor_copy(outt[0:1, 0:1], pS)
    # debug slots
    nc.vector.tensor_copy(outt[0:1, 1 : 1 + CL], tstar[0:1, :])
    nc.vector.tensor_copy(outt[0:1, 6 : 6 + CL], kq[0:1, :])
    nc.vector.tensor_copy(outt[0:1, 11 : 11 + CL], possum[0:1, :])
    nc.vector.tensor_copy(outt[0:1, 21 : 21 + CL], drate[0:1, :])
    nc.vector.tensor_copy(outt[0:1, 26 : 26 + CL], hardf[0:1, :])
    nc.sync.dma_start(out=out_d[:, :], in_=outt)
    ctx.close()


def build_nc():
    nc = bacc.Bacc("TRN2", target_bir_lowering=False, debug=False,
                   num_devices=NCORES)
    pred_d = nc.dram_tensor("pred", [CL, B], F32, kind="ExternalInput")
    targ_d = nc.dram_tensor("target", [CL, B], F32, kind="ExternalInput")
    rand_d = nc.dram_tensor("rand", [CL, B], F32, kind="ExternalInput")
    scal_d = nc.dram_tensor("scal", [CL, 2], F32, kind="ExternalInput")
    out_d = nc.dram_tensor("out", [1, NOUT], F32, kind="ExternalOutput")
    with tile.TileContext(nc) as tc:
        _body(tc, pred_d, targ_d, rand_d, scal_d, out_d)
    nc.compile()
    return nc


def _shard(pred, target, rand_mat, hard_rand, pos_prop):
    in_maps = []
    for i in range(NCORES):
        sl = slice(CL * i, CL * (i + 1))
        in_maps.append({
            "pred": np.ascontiguousarray(pred[:, sl].T),
            "target": np.ascontiguousarray(target[:, sl].T),
            "rand": np.ascontiguousarray(rand_mat[:, sl].T),
            "scal": np.ascontiguousarray(
                np.stack([pos_prop[sl], hard_rand[0, sl]], axis=1)
            ).astype(np.float32),
        })
    return in_maps


def _gather(res):
    total = sum(float(r["out"][0, 0]) for r in res.results)
    return np.array(total / (B * C), dtype=np.float32)


def kernel(pred, target, rand_mat, hard_rand, pos_prop):
    if "nc" not in _CACHED:
        _CACHED["nc"] = build_nc()
    nc = _CACHED["nc"]
    in_maps = _shard(pred, target, rand_mat, hard_rand, pos_prop)
    # the first exec after a fresh NEFF load occasionally dies with a
    # transient NRT_EXEC_UNIT_UNRECOVERABLE; one retry recovers it
    try:
        res = run_bass_kernel_spmd(nc, in_maps, core_ids=list(range(NCORES)))
    except Exception:
        import time as _time
        _time.sleep(2.0)
        res = run_bass_kernel_spmd(nc, in_maps, core_ids=list(range(NCORES)))
    return _gather(res)


def _install_ntff_shim():
    """The agent image's antenv lacks axon_hooks; shim it from trn_boot."""
    import sys, types
    try:
        from antenv import axon_hooks  # noqa: F401
        return
    except ImportError:
        pass
    import antenv
    from trn_agent_boot.trn_boot import _ntff_profile_via_ctypes
    hook = _ntff_profile_via_ctypes("/opt/axon/libaxon_pjrt.so")
    mod = types.ModuleType("antenv.axon_hooks")
    mod.get_axon_ntff_profile_hook = lambda: hook
    mod.set_axon_ntff_profile_hook = lambda h: None
    sys.modules["antenv.axon_hooks"] = mod
    antenv.axon_hooks = mod
    # artifact upload has no cloud access here; keep traces local
    import concourse.bass_utils as bu
    bu.upload_artifacts = lambda tmpdir: tmpdir


def kernel_profiled(pred, target, rand_mat, hard_rand, pos_prop, **trace_kwargs):
    """Same as kernel() but with NTFF tracing; returns (out, BassKernelResults)."""
    _install_ntff_shim()
    if "nc" not in _CACHED:
        _CACHED["nc"] = build_nc()
    nc = _CACHED["nc"]
    in_maps = _shard(pred, target, rand_mat, hard_rand, pos_prop)
    res = run_bass_kernel_spmd(nc, in_maps, core_ids=list(range(NCORES)),
                               trace=True, **trace_kwargs)
    return _gather(res), res


# revision 33
# speedup vs baseline: 1.0730x; 1.0400x over previous
"""Trainium2 Bass kernel for PosNegBalanceLoss.

Contract: kernel(**inputs) takes FULL unsharded inputs (pred/target/rand_mat
[131072,40] f32, hard_rand [1,40], pos_prop [40]) and returns the FULL scalar
output, distributing across 8 NeuronCores internally.

Sharding: by class columns (40 classes -> 5 per core). Each core gets
contiguous [5, 131072] slices (host-side transpose), so every per-class
reduction including the rank/argsort step is core-local. The only cross-core
step is an 8-byte AllReduce(max) of (ln_max, -ln_min) over per-class losses.

Math: with x2 = pred*(1-2*target), bce = logaddexp(0,pred) - pred*target
== softplus(x2), and g = |sigmoid(pred)-target| == sigmoid(x2), so the whole
elementwise phase is two DVE ops + two ACT table ops per class.

The rank step (drop the dropout_num smallest-g majority samples per class) is
a per-class threshold search: lockstep bisection with exact counts of
(g_masked < t) via ACT Sign+accum / DVE is_ge+accum; cross-partition count
totals are replicated with a PE double-matmul (acc @ ones -> [5,1];
broadcast-copy -> [5,128]; @ I5 -> [128,5]). Per-class scalars stay
replicated across partitions the whole time, so no partition broadcasts are
needed. A fixed number of bisection rounds leaves ~2e-5 relative error on the
final mean (only elements inside the final bisection interval can be
mis-dropped, each contributing ~2e-7).
"""

import numpy as np
from contextlib import ExitStack

import concourse.bass as bass
import concourse.tile as tile
from concourse import bacc, mybir
from concourse.bass_utils import run_bass_kernel_spmd

F32 = mybir.dt.float32
U8 = mybir.dt.uint8
ALU = mybir.AluOpType
AF = mybir.ActivationFunctionType

B = 131072            # batch (rows)
C = 40                # classes (cols)
NCORES = 8
CL = C // NCORES      # classes per core = 5
P = 128               # sbuf partitions
N = B // P            # 1024 elems per partition per class
BF = float(B)
BIS_ITERS = 8         # bisection rounds
NOUT = 32             # debug-friendly output vector per core

_CACHED = {}


def _bc(ap, n):
    """Broadcast a [p,1] AP along the free dim to [p,n] (stride-0)."""
    return bass.AP(tensor=ap.tensor, offset=ap.offset, ap=[ap.ap[0], [0, n]])


def _body(tc: tile.TileContext, pred_d, targ_d, rand_d, scal_d, out_d):
    nc = tc.nc
    ctx = ExitStack()
    main = ctx.enter_context(tc.tile_pool(name="main", bufs=1))
    tmp3 = ctx.enter_context(tc.tile_pool(name="tmp3", bufs=2))
    psum = ctx.enter_context(tc.tile_pool(name="psum", bufs=2, space="PSUM"))
    dram = ctx.enter_context(tc.tile_pool(name="dram", bufs=1, space="DRAM"))

    # ---------------- persistent per-class tiles ----------------
    targT = [main.tile([P, N], F32, tag=f"targ{c}", name=f"targ{c}") for c in range(CL)]
    randT = [main.tile([P, N], F32, tag=f"rand{c}", name=f"rand{c}") for c in range(CL)]
    bceT = [main.tile([P, N], F32, tag=f"bce{c}", name=f"bce{c}") for c in range(CL)]
    gT = [main.tile([P, N], F32, tag=f"g{c}", name=f"g{c}") for c in range(CL)]
    gqT = [main.tile([P, N], mybir.dt.bfloat16, tag=f"gq{c}", name=f"gq{c}") for c in range(CL)]
    x2T = [main.tile([P, N], F32, tag=f"x2{c}", name=f"x2{c}") for c in range(CL)]

    # replicated per-class scalar tiles [P, CL] and accumulators
    accE = main.tile([P, CL], F32, tag="accE", name="accE")   # sum(1-2t) partials
    accL = main.tile([P, CL], F32, tag="accL", name="accL")   # sum(bce) partials
    scal_sb = main.tile([P, CL, 2], F32, tag="scal_sb", name="scal_sb")

    junkA = main.tile([P, N], F32, tag="junkA", name="junkA")
    junkD = main.tile([P, N], F32, tag="junkD", name="junkD")
    ones_n = main.tile([P, N], F32, tag="ones_n", name="ones_n")
    nc.vector.memset(ones_n, 1.0)
    ones_col = main.tile([P, 1], F32, tag="ones_col", name="ones_col")
    nc.vector.memset(ones_col, 1.0)
    n08_col = main.tile([P, 1], F32, tag="n08_col", name="n08_col")
    nc.vector.memset(n08_col, -0.8)
    ones5 = main.tile([CL, 1], F32, tag="ones5", name="ones5")
    nc.vector.memset(ones5, 1.0)
    I5 = main.tile([CL, CL], F32, tag="I5", name="I5")
    I5_d = nc.inline_tensor(np.eye(CL, dtype=np.float32), name="I5c")
    nc.sync.dma_start(out=I5, in_=I5_d.ap())

    def repl_reduce(acc, nm):
        """[P, CL] partials -> PSUM [P, CL] with per-class totals replicated."""
        p1 = psum.tile([CL, 1], F32, tag="p1", name=f"p1_{nm}")
        nc.tensor.matmul(out=p1, lhsT=acc, rhs=ones_col, start=True, stop=True)
        s1 = main.tile([CL, P], F32, tag="s1r", name=f"s1_{nm}")
        nc.vector.tensor_copy(s1, _bc(p1, P))
        p2 = psum.tile([P, CL], F32, tag="p2", name=f"p2_{nm}")
        nc.tensor.matmul(out=p2, lhsT=s1, rhs=I5, start=True, stop=True)
        return p2

    # ---------------- DMA in ----------------
    pred_ap = pred_d.ap()
    targ_ap = targ_d.ap()
    rand_ap = rand_d.ap()

    predT = []
    for c in range(CL):
        pt = tmp3.tile([P, N], F32, tag="t0", name="pred")
        nc.sync.dma_start(
            out=pt, in_=pred_ap[c : c + 1, :].rearrange("a (p n) -> (a p) n", p=P)
        )
        predT.append(pt)
        nc.sync.dma_start(
            out=targT[c],
            in_=targ_ap[c : c + 1, :].rearrange("a (p n) -> (a p) n", p=P),
        )
    sc_ap = scal_d.ap()
    sc_b = bass.AP(
        tensor=sc_ap.tensor, offset=sc_ap.offset, ap=[[0, P], sc_ap.ap[0], sc_ap.ap[1]]
    )
    nc.sync.dma_start(out=scal_sb, in_=sc_b)
    for c in range(CL):
        nc.sync.dma_start(
            out=randT[c],
            in_=rand_ap[c : c + 1, :].rearrange("a (p n) -> (a p) n", p=P),
        )

    # ---------------- phase E ----------------
    # x2 = pred*(1-2t); bce = 0.5*(x2+|x2|)+ln(1+exp(-|x2|)); g = sigmoid(x2)
    for c in range(CL):
        s2 = tmp3.tile([P, N], F32, tag="t1", name="s2")
        nc.vector.scalar_tensor_tensor(
            out=s2, in0=targT[c], scalar=-2.0, in1=ones_n,
            op0=ALU.mult, op1=ALU.add, accum_out=accE[:, c : c + 1],
        )
        nc.vector.tensor_mul(x2T[c], predT[c], s2)
        abits = tmp3.tile([P, N], F32, tag="t3", name="abits")
        nc.vector.tensor_scalar(out=abits.bitcast(mybir.dt.uint32),
                                in0=x2T[c].bitcast(mybir.dt.uint32),
                                scalar1=0x7FFFFFFF, scalar2=None, op0=ALU.bitwise_and)
        el = tmp3.tile([P, N], F32, tag="t4", name="el")
        nc.scalar.activation(out=el, in_=abits, func=AF.Exp, scale=-1.0)
        nc.scalar.activation(out=el, in_=el, func=AF.Ln, bias=1.0)
        r2v = tmp3.tile([P, N], F32, tag="t5", name="r2v")
        nc.gpsimd.tensor_tensor(out=r2v, in0=x2T[c], in1=abits, op=ALU.add)
        nc.vector.scalar_tensor_tensor(
            out=bceT[c], in0=r2v, scalar=0.5, in1=el,
            op0=ALU.mult, op1=ALU.add, accum_out=accL[:, c : c + 1],
        )
    for c in range(CL):
        nc.scalar.activation(out=gT[c], in_=x2T[c], func=AF.Sigmoid)

    # ---------------- per-class scalars (replicated [P, CL]) ----------------
    S2R = repl_reduce(accE, "S2R")
    possum = main.tile([P, CL], F32, tag="possum", name="possum")
    nc.vector.tensor_scalar(out=possum, in0=S2R, scalar1=-0.5, scalar2=BF / 2,
                            op0=ALU.mult, op1=ALU.add)
    balpos = main.tile([P, CL], F32, tag="balpos", name="balpos")
    nc.vector.tensor_scalar(out=balpos, in0=scal_sb[:, :, 0], scalar1=BF, scalar2=None,
                            op0=ALU.mult)
    posgt = main.tile([P, CL], F32, tag="posgt", name="posgt")
    nc.vector.tensor_tensor(out=posgt, in0=possum, in1=balpos, op=ALU.is_gt)
    posgt_m = main.tile([P, CL], U8, tag="posgt_m", name="posgt_m")
    nc.vector.tensor_tensor(out=posgt_m, in0=possum, in1=balpos, op=ALU.is_gt)
    kq = main.tile([P, CL], F32, tag="kq", name="kq")
    nc.vector.tensor_sub(kq, possum, balpos)
    nc.vector.tensor_scalar(out=kq.bitcast(mybir.dt.uint32), in0=kq.bitcast(mybir.dt.uint32),
                            scalar1=0x7FFFFFFF, scalar2=None, op0=ALU.bitwise_and)
    # notm = (target != majlab) = target*(1-2*posgt) + posgt
    u2 = main.tile([P, CL], F32, tag="u2", name="u2")
    nc.vector.tensor_scalar(out=u2, in0=posgt, scalar1=-2.0, scalar2=1.0,
                            op0=ALU.mult, op1=ALU.add)
    # balance = posgt ? balpos : B-balpos
    balance = main.tile([P, CL], F32, tag="balance", name="balance")
    nc.vector.tensor_scalar(out=balance, in0=balpos, scalar1=-1.0, scalar2=BF,
                            op0=ALU.mult, op1=ALU.add)
    nc.vector.copy_predicated(out=balance, mask=posgt_m, data=balpos)
    # majcnt = posgt ? possum : B-possum ; mincnt = B-majcnt
    majcnt = main.tile([P, CL], F32, tag="majcnt", name="majcnt")
    nc.vector.tensor_scalar(out=majcnt, in0=possum, scalar1=-1.0, scalar2=BF,
                            op0=ALU.mult, op1=ALU.add)
    nc.vector.copy_predicated(out=majcnt, mask=posgt_m, data=possum)
    mincnt = main.tile([P, CL], F32, tag="mincnt", name="mincnt")
    nc.vector.tensor_scalar(out=mincnt, in0=majcnt, scalar1=-1.0, scalar2=BF,
                            op0=ALU.mult, op1=ALU.add)
    rmaj = main.tile([P, CL], F32, tag="rmaj", name="rmaj")
    nc.vector.reciprocal(out=rmaj, in_=majcnt)
    rmin = main.tile([P, CL], F32, tag="rmin", name="rmin")
    nc.vector.reciprocal(out=rmin, in_=mincnt)
    alpha = main.tile([P, CL], F32, tag="alpha", name="alpha")
    nc.vector.tensor_mul(alpha, balance, rmaj)
    bminus = main.tile([P, CL], F32, tag="bminus", name="bminus")
    nc.vector.tensor_scalar(out=bminus, in0=balance, scalar1=-1.0, scalar2=BF,
                            op0=ALU.mult, op1=ALU.add)
    beta = main.tile([P, CL], F32, tag="beta", name="beta")
    nc.vector.tensor_mul(beta, bminus, rmin)

    # gmb = 2*notm + g in bf16: maj samples in (0,1), min samples in (2,3)
    for c in range(CL):
        notm = tmp3.tile([P, N], F32, tag="t2", name="notm")
        nc.vector.tensor_scalar(out=notm, in0=targT[c], scalar1=u2[:, c : c + 1],
                                scalar2=posgt[:, c : c + 1], op0=ALU.mult, op1=ALU.add)
        nc.vector.scalar_tensor_tensor(
            out=gqT[c], in0=notm, scalar=2.0, in1=gT[c],
            op0=ALU.mult, op1=ALU.add,
        )

    # ---------------- loss_c -> ln -> collective max ----------------
    LR = repl_reduce(accL, "LR")
    lnT = main.tile([P, CL], F32, tag="lnT", name="lnT")
    nc.scalar.activation(out=lnT, in_=LR, func=AF.Ln, bias=1.0)
    pair = main.tile([P, 2], F32, tag="pair", name="pair")
    nc.vector.tensor_reduce(out=pair[:, 0:1], in_=lnT, axis=mybir.AxisListType.X,
                            op=ALU.max)
    negln = main.tile([P, CL], F32, tag="negln", name="negln")
    nc.vector.tensor_scalar(out=negln, in0=lnT, scalar1=-1.0, scalar2=None, op0=ALU.mult)
    nc.vector.tensor_reduce(out=pair[:, 1:2], in_=negln, axis=mybir.AxisListType.X,
                            op=ALU.max)
    cc_in = dram.tile([1, 2], F32, name="cc_in")
    cc_out = dram.tile([1, 2], F32, name="cc_out")
    nc.sync.dma_start(out=cc_in, in_=pair[0:1, :])
    nc.gpsimd.collective_compute(
        "AllReduce", ALU.max,
        replica_groups=[list(range(NCORES))],
        ins=[cc_in[:].opt()], outs=[cc_out[:].opt()],
    )
    mx = main.tile([P, 2], F32, tag="mx", name="mx")
    cc_out_ap = cc_out[:]
    mx_b = bass.AP(tensor=cc_out_ap.tensor, offset=cc_out_ap.offset,
                   ap=[[0, P], cc_out_ap.ap[1]])
    nc.sync.dma_start(out=mx, in_=mx_b)

    # drate = sigmoid(5 - 10*(ln - lnmin)/(lnmax - lnmin))
    den = main.tile([P, 1], F32, tag="den", name="den")
    nc.vector.tensor_add(den, mx[:, 0:1], mx[:, 1:2])
    rden = main.tile([P, 1], F32, tag="rden", name="rden")
    nc.vector.reciprocal(out=rden, in_=den)
    num = main.tile([P, CL], F32, tag="num", name="num")
    nc.vector.tensor_add(num, lnT, _bc(mx[:, 1:2], CL))
    nc.vector.tensor_mul(num, num, _bc(rden, CL))
    norm = main.tile([P, CL], F32, tag="norm", name="norm")
    nc.vector.tensor_scalar(out=norm, in0=num, scalar1=-10.0, scalar2=5.0,
                            op0=ALU.mult, op1=ALU.add)
    drate = main.tile([P, CL], F32, tag="drate", name="drate")
    nc.scalar.activation(out=drate, in_=norm, func=AF.Sigmoid)
    ndrate = main.tile([P, CL], F32, tag="ndrate", name="ndrate")
    nc.vector.tensor_scalar(out=ndrate, in0=drate, scalar1=-1.0, scalar2=None,
                            op0=ALU.mult)
    hardf = main.tile([P, CL], U8, tag="hardf", name="hardf")
    nc.vector.tensor_tensor(out=hardf, in0=scal_sb[:, :, 1], in1=drate, op=ALU.is_gt)

    # folded weight scalars: w = target*A2 + B2, then -0.5 for the sign trick
    am1 = main.tile([P, CL], F32, tag="am1", name="am1")
    nc.vector.tensor_scalar(out=am1, in0=alpha, scalar1=-1.0, scalar2=None, op0=ALU.add)
    Aw = main.tile([P, CL], F32, tag="Aw", name="Aw")
    nc.vector.tensor_scalar(out=Aw, in0=beta, scalar1=-1.0, scalar2=1.0,
                            op0=ALU.mult, op1=ALU.add)
    nc.vector.copy_predicated(out=Aw, mask=hardf, data=am1)
    ones = main.tile([P, CL], F32, tag="ones", name="ones")
    nc.vector.memset(ones, 1.0)
    Bw = main.tile([P, CL], F32, tag="Bw", name="Bw")
    nc.vector.tensor_copy(Bw, beta)
    nc.vector.copy_predicated(out=Bw, mask=hardf, data=ones)
    A2 = main.tile([P, CL], F32, tag="A2", name="A2")
    nc.vector.tensor_mul(A2, u2, Aw)
    B2s = main.tile([P, CL], F32, tag="B2s", name="B2s")
    nc.vector.tensor_mul(B2s, posgt, Aw)
    nc.vector.tensor_add(B2s, B2s, Bw)
    nc.vector.tensor_scalar(out=B2s, in0=B2s, scalar1=-0.5, scalar2=None, op0=ALU.add)

    # ---------------- bisection for per-class k-th smallest ----------------
    loE = main.tile([P, CL], F32, tag="loE", name="loE")
    nc.vector.memset(loE, 0.0)
    thr = main.tile([P, CL], F32, tag="thr", name="thr")
    # ACT classes (0..2): stat = sum sign(te-gmb) = 2cnt-B -> thr = 2k-B;
    # DVE classes (3..4): stat = cnt -> thr = k. flag_hi = stat > thr.
    nc.vector.tensor_copy(thr, kq)
    nc.vector.tensor_scalar(out=thr[:, 0:2], in0=kq[:, 0:2], scalar1=2.0, scalar2=-BF,
                            op0=ALU.mult, op1=ALU.add)
    bacc_t = main.tile([P, CL], F32, tag="bacc", name="bacc")
    te = main.tile([P, CL], F32, tag="te", name="te")
    flo_f = main.tile([P, CL], F32, tag="flo_f", name="flo_f")

    for it in range(BIS_ITERS):
        w_half = 2.0 ** -(it + 1)
        nc.vector.tensor_scalar(out=te, in0=loE, scalar1=w_half, scalar2=None,
                                op0=ALU.add)
        for c in range(2):
            nc.scalar.activation(out=junkA, in_=gqT[c], func=AF.Sign,
                                 scale=-1.0, bias=te[:, c : c + 1],
                                 accum_out=bacc_t[:, c : c + 1])
        for c in range(2, CL):
            nc.vector.tensor_scalar(out=junkD, in0=gqT[c], scalar1=te[:, c : c + 1],
                                    scalar2=None, op0=ALU.is_lt, op1=ALU.add,
                                    accum_out=bacc_t[:, c : c + 1])
        bRp = repl_reduce(bacc_t, f"bR{it}")
        nc.vector.tensor_tensor(out=flo_f, in0=bRp, in1=thr, op=ALU.is_le)
        nc.vector.scalar_tensor_tensor(out=loE, in0=flo_f, scalar=w_half, in1=loE,
                                       op0=ALU.mult, op1=ALU.add)

    # final threshold (already offset); hard classes get sentinel -9
    tadj = main.tile([P, CL], F32, tag="tadj", name="tadj")
    nc.vector.tensor_scalar(out=tadj, in0=loE, scalar1=2.0 ** -(BIS_ITERS + 1),
                            scalar2=None, op0=ALU.add)
    tstar = main.tile([P, CL], F32, tag="tstar", name="tstar")
    nc.vector.tensor_copy(tstar, tadj)
    neg9 = main.tile([P, CL], F32, tag="neg9", name="neg9")
    nc.vector.memset(neg9, -9.0)
    nc.vector.copy_predicated(out=tadj, mask=hardf, data=neg9)
    ntadj = main.tile([P, CL], F32, tag="ntadj", name="ntadj")
    nc.vector.tensor_scalar(out=ntadj, in0=tadj, scalar1=-1.0, scalar2=None,
                            op0=ALU.mult)

    # ---------------- final weighted sum ----------------
    # m1 = bce*(1 - di*ix) with di = (rand>drate), ix = (g>=0.8);
    # total_c = sum(m1*w1) + 0.5*sum(m1*sign(gq-tadj))
    f1acc = main.tile([P, CL], F32, tag="f1acc", name="f1acc")
    f2acc = main.tile([P, CL], F32, tag="f2acc", name="f2acc")
    m1T = []
    for c in range(CL):
        sF = tmp3.tile([P, N], F32, tag="t2", name="sF")
        nc.scalar.activation(out=sF, in_=randT[c], func=AF.Sign,
                             bias=ndrate[:, c : c + 1])
        six = tmp3.tile([P, N], F32, tag="t3", name="six")
        nc.scalar.activation(out=six, in_=gT[c], func=AF.Sign, bias=n08_col)
        ssum = tmp3.tile([P, N], F32, tag="t4", name="ssum")
        nc.vector.tensor_add(ssum, sF, six)
        nfi = tmp3.tile([P, N], F32, tag="t5", name="nfi")
        nc.vector.tensor_scalar(out=nfi, in0=ssum, scalar1=1.9, scalar2=None,
                                op0=ALU.is_lt)
        m1 = tmp3.tile([P, N], F32, tag="t6", name="m1", bufs=6)
        nc.vector.tensor_mul(m1, bceT[c], nfi)
        m1T.append(m1)
        w1 = tmp3.tile([P, N], F32, tag="t1", name="w1")
        nc.scalar.activation(out=w1, in_=targT[c], func=AF.Identity,
                             scale=A2[:, c : c + 1], bias=B2s[:, c : c + 1])
        nc.vector.scalar_tensor_tensor(out=junkD, in0=m1, scalar=1.0, in1=w1,
                                       op0=ALU.mult, op1=ALU.mult,
                                       accum_out=f1acc[:, c : c + 1])
    for c in range(CL):
        sg = tmp3.tile([P, N], F32, tag="t0", name="sg")
        nc.scalar.activation(out=sg, in_=gqT[c], func=AF.Sign,
                             bias=ntadj[:, c : c + 1])
        nc.vector.scalar_tensor_tensor(out=junkD, in0=m1T[c], scalar=1.0, in1=sg,
                                       op0=ALU.mult, op1=ALU.mult,
                                       accum_out=f2acc[:, c : c + 1])

    diff = main.tile([P, CL], F32, tag="diff", name="diff")
    nc.vector.scalar_tensor_tensor(out=diff, in0=f2acc, scalar=0.5, in1=f1acc,
                                   op0=ALU.mult, op1=ALU.add)
    # total = sum over partitions and classes via two tiny matmuls
    pT = psum.tile([CL, 1], F32, tag="p1", name="pT")
    nc.tensor.matmul(out=pT, lhsT=diff, rhs=ones_col, start=True, stop=True)
    s5 = main.tile([CL, 1], F32, tag="s5", name="s5")
    nc.vector.tensor_copy(s5, pT)
    pS = psum.tile([1, 1], F32, tag="p2", name="pS")
    nc.tensor.matmul(out=pS, lhsT=s5, rhs=ones5, start=True, stop=True)

    outt = main.tile([1, NOUT], F32, tag="outt", name="outt")
    nc.vector.memset(outt, 0.0)
    nc.vector.tensor_copy(outt[0:1, 0:1], pS)
    # debug slots
    nc.vector.tensor_copy(outt[0:1, 1 : 1 + CL], tstar[0:1, :])
    nc.vector.tensor_copy(outt[0:1, 6 : 6 + CL], kq[0:1, :])
    nc.vector.tensor_copy(outt[0:1, 11 : 11 + CL], possum[0:1, :])
    nc.vector.tensor_copy(outt[0:1, 21 : 21 + CL], drate[0:1, :])
    nc.vector.tensor_copy(outt[0:1, 26 : 26 + CL], hardf[0:1, :])
    nc.sync.dma_start(out=out_d[:, :], in_=outt)
    ctx.close()


def build_nc():
    nc = bacc.Bacc("TRN2", target_bir_lowering=False, debug=False,
                   num_devices=NCORES)
    pred_d = nc.dram_tensor("pred", [CL, B], F32, kind="ExternalInput")
    targ_d = nc.dram_tensor("target", [CL, B], F32, kind="ExternalInput")
    rand_d = nc.dram_tensor("rand", [CL, B], F32, kind="ExternalInput")
    scal_d = nc.dram_tensor("scal", [CL, 2], F32, kind="ExternalInput")
    out_d = nc.dram_tensor("out", [1, NOUT], F32, kind="ExternalOutput")
    with tile.TileContext(nc) as tc:
        _body(tc, pred_d, targ_d, rand_d, scal_d, out_d)
    nc.compile()
    return nc


def _shard(pred, target, rand_mat, hard_rand, pos_prop):
    in_maps = []
    for i in range(NCORES):
        sl = slice(CL * i, CL * (i + 1))
        in_maps.append({
            "pred": np.ascontiguousarray(pred[:, sl].T),
            "target": np.ascontiguousarray(target[:, sl].T),
            "rand": np.ascontiguousarray(rand_mat[:, sl].T),
            "scal": np.ascontiguousarray(
                np.stack([pos_prop[sl], hard_rand[0, sl]], axis=1)
            ).astype(np.float32),
        })
    return in_maps


def _gather(res):
    total = sum(float(r["out"][0, 0]) for r in res.results)
    return np.array(total / (B * C), dtype=np.float32)


def kernel(pred, target, rand_mat, hard_rand, pos_prop):
    if "nc" not in _CACHED:
        _CACHED["nc"] = build_nc()
    nc = _CACHED["nc"]
    in_maps = _shard(pred, target, rand_mat, hard_rand, pos_prop)
    # the first exec after a fresh NEFF load occasionally dies with a
    # transient NRT_EXEC_UNIT_UNRECOVERABLE; one retry recovers it
    try:
        res = run_bass_kernel_spmd(nc, in_maps, core_ids=list(range(NCORES)))
    except Exception:
        import time as _time
        _time.sleep(2.0)
        res = run_bass_kernel_spmd(nc, in_maps, core_ids=list(range(NCORES)))
    return _gather(res)


def _install_ntff_shim():
    """The agent image's antenv lacks axon_hooks; shim it from trn_boot."""
    import sys, types
    try:
        from antenv import axon_hooks  # noqa: F401
        return
    except ImportError:
        pass
    import antenv
    from trn_agent_boot.trn_boot import _ntff_profile_via_ctypes
    hook = _ntff_profile_via_ctypes("/opt/axon/libaxon_pjrt.so")
    mod = types.ModuleType("antenv.axon_hooks")
    mod.get_axon_ntff_profile_hook = lambda: hook
    mod.set_axon_ntff_profile_hook = lambda h: None
    sys.modules["antenv.axon_hooks"] = mod
    antenv.axon_hooks = mod
    # artifact upload has no cloud access here; keep traces local
    import concourse.bass_utils as bu
    bu.upload_artifacts = lambda tmpdir: tmpdir


def kernel_profiled(pred, target, rand_mat, hard_rand, pos_prop, **trace_kwargs):
    """Same as kernel() but with NTFF tracing; returns (out, BassKernelResults)."""
    _install_ntff_shim()
    if "nc" not in _CACHED:
        _CACHED["nc"] = build_nc()
    nc = _CACHED["nc"]
    in_maps = _shard(pred, target, rand_mat, hard_rand, pos_prop)
    res = run_bass_kernel_spmd(nc, in_maps, core_ids=list(range(NCORES)),
                               trace=True, **trace_kwargs)
    return _gather(res), res


# revision 34
# speedup vs baseline: 1.0993x; 1.0246x over previous
"""Trainium2 Bass kernel for PosNegBalanceLoss.

Contract: kernel(**inputs) takes FULL unsharded inputs (pred/target/rand_mat
[131072,40] f32, hard_rand [1,40], pos_prop [40]) and returns the FULL scalar
output, distributing across 8 NeuronCores internally.

Sharding: by class columns (40 classes -> 5 per core). Each core gets
contiguous [5, 131072] slices (host-side transpose), so every per-class
reduction including the rank/argsort step is core-local. The only cross-core
step is an 8-byte AllReduce(max) of (ln_max, -ln_min) over per-class losses.

Math: with x2 = pred*(1-2*target), bce = logaddexp(0,pred) - pred*target
== softplus(x2), and g = |sigmoid(pred)-target| == sigmoid(x2), so the whole
elementwise phase is two DVE ops + two ACT table ops per class.

The rank step (drop the dropout_num smallest-g majority samples per class) is
a per-class threshold search: lockstep bisection with exact counts of
(g_masked < t) via ACT Sign+accum / DVE is_ge+accum; cross-partition count
totals are replicated with a PE double-matmul (acc @ ones -> [5,1];
broadcast-copy -> [5,128]; @ I5 -> [128,5]). Per-class scalars stay
replicated across partitions the whole time, so no partition broadcasts are
needed. A fixed number of bisection rounds leaves ~2e-5 relative error on the
final mean (only elements inside the final bisection interval can be
mis-dropped, each contributing ~2e-7).
"""

import numpy as np
from contextlib import ExitStack

import concourse.bass as bass
import concourse.tile as tile
from concourse import bacc, mybir
from concourse.bass_utils import run_bass_kernel_spmd

F32 = mybir.dt.float32
U8 = mybir.dt.uint8
ALU = mybir.AluOpType
AF = mybir.ActivationFunctionType

B = 131072            # batch (rows)
C = 40                # classes (cols)
NCORES = 8
CL = C // NCORES      # classes per core = 5
P = 128               # sbuf partitions
N = B // P            # 1024 elems per partition per class
BF = float(B)
BIS_ITERS = 8         # bisection rounds
NOUT = 32             # debug-friendly output vector per core

_CACHED = {}


def _bc(ap, n):
    """Broadcast a [p,1] AP along the free dim to [p,n] (stride-0)."""
    return bass.AP(tensor=ap.tensor, offset=ap.offset, ap=[ap.ap[0], [0, n]])


def _body(tc: tile.TileContext, pred_d, targ_d, rand_d, scal_d, out_d):
    nc = tc.nc
    ctx = ExitStack()
    main = ctx.enter_context(tc.tile_pool(name="main", bufs=1))
    tmp3 = ctx.enter_context(tc.tile_pool(name="tmp3", bufs=2))
    psum = ctx.enter_context(tc.tile_pool(name="psum", bufs=2, space="PSUM"))
    dram = ctx.enter_context(tc.tile_pool(name="dram", bufs=1, space="DRAM"))

    # ---------------- persistent per-class tiles ----------------
    targT = [main.tile([P, N], F32, tag=f"targ{c}", name=f"targ{c}") for c in range(CL)]
    randT = [main.tile([P, N], F32, tag=f"rand{c}", name=f"rand{c}") for c in range(CL)]
    bceT = [main.tile([P, N], F32, tag=f"bce{c}", name=f"bce{c}") for c in range(CL)]
    gT = [main.tile([P, N], F32, tag=f"g{c}", name=f"g{c}") for c in range(CL)]
    gqT = [main.tile([P, N], mybir.dt.bfloat16, tag=f"gq{c}", name=f"gq{c}") for c in range(CL)]
    x2T = [main.tile([P, N], F32, tag=f"x2{c}", name=f"x2{c}") for c in range(CL)]

    # replicated per-class scalar tiles [P, CL] and accumulators
    accE = main.tile([P, CL], F32, tag="accE", name="accE")   # sum(1-2t) partials
    accL = main.tile([P, CL], F32, tag="accL", name="accL")   # sum(bce) partials
    scal_sb = main.tile([P, CL, 2], F32, tag="scal_sb", name="scal_sb")

    junkA = main.tile([P, N], F32, tag="junkA", name="junkA")
    junkD = main.tile([P, N], F32, tag="junkD", name="junkD")
    ones_n = main.tile([P, N], F32, tag="ones_n", name="ones_n")
    nc.vector.memset(ones_n, 1.0)
    ones_col = main.tile([P, 1], F32, tag="ones_col", name="ones_col")
    nc.vector.memset(ones_col, 1.0)
    n08_col = main.tile([P, 1], F32, tag="n08_col", name="n08_col")
    nc.vector.memset(n08_col, -0.8)
    ones5 = main.tile([CL, 1], F32, tag="ones5", name="ones5")
    nc.vector.memset(ones5, 1.0)
    I5 = main.tile([CL, CL], F32, tag="I5", name="I5")
    I5_d = nc.inline_tensor(np.eye(CL, dtype=np.float32), name="I5c")
    nc.sync.dma_start(out=I5, in_=I5_d.ap())

    def repl_reduce(acc, nm):
        """[P, CL] partials -> PSUM [P, CL] with per-class totals replicated."""
        p1 = psum.tile([CL, 1], F32, tag="p1", name=f"p1_{nm}")
        nc.tensor.matmul(out=p1, lhsT=acc, rhs=ones_col, start=True, stop=True)
        s1 = main.tile([CL, P], F32, tag="s1r", name=f"s1_{nm}")
        nc.vector.tensor_copy(s1, _bc(p1, P))
        p2 = psum.tile([P, CL], F32, tag="p2", name=f"p2_{nm}")
        nc.tensor.matmul(out=p2, lhsT=s1, rhs=I5, start=True, stop=True)
        return p2

    # ---------------- DMA in ----------------
    pred_ap = pred_d.ap()
    targ_ap = targ_d.ap()
    rand_ap = rand_d.ap()

    predT = []
    for c in range(CL):
        pt = tmp3.tile([P, N], F32, tag="t0", name="pred")
        nc.sync.dma_start(
            out=pt, in_=pred_ap[c : c + 1, :].rearrange("a (p n) -> (a p) n", p=P)
        )
        predT.append(pt)
        nc.sync.dma_start(
            out=targT[c],
            in_=targ_ap[c : c + 1, :].rearrange("a (p n) -> (a p) n", p=P),
        )
    sc_ap = scal_d.ap()
    sc_b = bass.AP(
        tensor=sc_ap.tensor, offset=sc_ap.offset, ap=[[0, P], sc_ap.ap[0], sc_ap.ap[1]]
    )
    nc.sync.dma_start(out=scal_sb, in_=sc_b)
    for c in range(CL):
        nc.sync.dma_start(
            out=randT[c],
            in_=rand_ap[c : c + 1, :].rearrange("a (p n) -> (a p) n", p=P),
        )

    # ---------------- phase E ----------------
    # x2 = pred*(1-2t); bce = 0.5*(x2+|x2|)+ln(1+exp(-|x2|)); g = sigmoid(x2)
    for c in range(CL):
        s2 = tmp3.tile([P, N], F32, tag="t1", name="s2")
        nc.vector.scalar_tensor_tensor(
            out=s2, in0=targT[c], scalar=-2.0, in1=ones_n,
            op0=ALU.mult, op1=ALU.add, accum_out=accE[:, c : c + 1],
        )
        nc.vector.tensor_mul(x2T[c], predT[c], s2)
        abits = tmp3.tile([P, N], F32, tag="t3", name="abits")
        nc.vector.tensor_scalar(out=abits.bitcast(mybir.dt.uint32),
                                in0=x2T[c].bitcast(mybir.dt.uint32),
                                scalar1=0x7FFFFFFF, scalar2=None, op0=ALU.bitwise_and)
        el = tmp3.tile([P, N], F32, tag="t4", name="el")
        nc.scalar.activation(out=el, in_=abits, func=AF.Exp, scale=-1.0)
        nc.scalar.activation(out=el, in_=el, func=AF.Ln, bias=1.0)
        r2v = tmp3.tile([P, N], F32, tag="t5", name="r2v")
        nc.gpsimd.tensor_tensor(out=r2v, in0=x2T[c], in1=abits, op=ALU.add)
        nc.vector.scalar_tensor_tensor(
            out=bceT[c], in0=r2v, scalar=0.5, in1=el,
            op0=ALU.mult, op1=ALU.add, accum_out=accL[:, c : c + 1],
        )
    for c in range(CL):
        nc.scalar.activation(out=gT[c], in_=x2T[c], func=AF.Sigmoid)

    # ---------------- per-class scalars (replicated [P, CL]) ----------------
    S2R = repl_reduce(accE, "S2R")
    possum = main.tile([P, CL], F32, tag="possum", name="possum")
    nc.vector.tensor_scalar(out=possum, in0=S2R, scalar1=-0.5, scalar2=BF / 2,
                            op0=ALU.mult, op1=ALU.add)
    balpos = main.tile([P, CL], F32, tag="balpos", name="balpos")
    nc.vector.tensor_scalar(out=balpos, in0=scal_sb[:, :, 0], scalar1=BF, scalar2=None,
                            op0=ALU.mult)
    posgt = main.tile([P, CL], F32, tag="posgt", name="posgt")
    nc.vector.tensor_tensor(out=posgt, in0=possum, in1=balpos, op=ALU.is_gt)
    posgt_m = main.tile([P, CL], U8, tag="posgt_m", name="posgt_m")
    nc.vector.tensor_tensor(out=posgt_m, in0=possum, in1=balpos, op=ALU.is_gt)
    kq = main.tile([P, CL], F32, tag="kq", name="kq")
    nc.vector.tensor_sub(kq, possum, balpos)
    nc.vector.tensor_scalar(out=kq.bitcast(mybir.dt.uint32), in0=kq.bitcast(mybir.dt.uint32),
                            scalar1=0x7FFFFFFF, scalar2=None, op0=ALU.bitwise_and)
    # notm = (target != majlab) = target*(1-2*posgt) + posgt
    u2 = main.tile([P, CL], F32, tag="u2", name="u2")
    nc.vector.tensor_scalar(out=u2, in0=posgt, scalar1=-2.0, scalar2=1.0,
                            op0=ALU.mult, op1=ALU.add)
    # balance = posgt ? balpos : B-balpos
    balance = main.tile([P, CL], F32, tag="balance", name="balance")
    nc.vector.tensor_scalar(out=balance, in0=balpos, scalar1=-1.0, scalar2=BF,
                            op0=ALU.mult, op1=ALU.add)
    nc.vector.copy_predicated(out=balance, mask=posgt_m, data=balpos)
    # majcnt = posgt ? possum : B-possum ; mincnt = B-majcnt
    majcnt = main.tile([P, CL], F32, tag="majcnt", name="majcnt")
    nc.vector.tensor_scalar(out=majcnt, in0=possum, scalar1=-1.0, scalar2=BF,
                            op0=ALU.mult, op1=ALU.add)
    nc.vector.copy_predicated(out=majcnt, mask=posgt_m, data=possum)
    mincnt = main.tile([P, CL], F32, tag="mincnt", name="mincnt")
    nc.vector.tensor_scalar(out=mincnt, in0=majcnt, scalar1=-1.0, scalar2=BF,
                            op0=ALU.mult, op1=ALU.add)
    rmaj = main.tile([P, CL], F32, tag="rmaj", name="rmaj")
    nc.vector.reciprocal(out=rmaj, in_=majcnt)
    rmin = main.tile([P, CL], F32, tag="rmin", name="rmin")
    nc.vector.reciprocal(out=rmin, in_=mincnt)
    alpha = main.tile([P, CL], F32, tag="alpha", name="alpha")
    nc.vector.tensor_mul(alpha, balance, rmaj)
    bminus = main.tile([P, CL], F32, tag="bminus", name="bminus")
    nc.vector.tensor_scalar(out=bminus, in0=balance, scalar1=-1.0, scalar2=BF,
                            op0=ALU.mult, op1=ALU.add)
    beta = main.tile([P, CL], F32, tag="beta", name="beta")
    nc.vector.tensor_mul(beta, bminus, rmin)

    # gmb = 2*notm + g in bf16: maj samples in (0,1), min samples in (2,3)
    for c in range(CL):
        notm = tmp3.tile([P, N], F32, tag="t2", name="notm")
        nc.vector.tensor_scalar(out=notm, in0=targT[c], scalar1=u2[:, c : c + 1],
                                scalar2=posgt[:, c : c + 1], op0=ALU.mult, op1=ALU.add)
        nc.vector.scalar_tensor_tensor(
            out=gqT[c], in0=notm, scalar=2.0, in1=gT[c],
            op0=ALU.mult, op1=ALU.add,
        )

    # ---------------- loss_c -> ln -> collective max ----------------
    LR = repl_reduce(accL, "LR")
    lnT = main.tile([P, CL], F32, tag="lnT", name="lnT")
    nc.scalar.activation(out=lnT, in_=LR, func=AF.Ln, bias=1.0)
    pair = main.tile([P, 2], F32, tag="pair", name="pair")
    nc.vector.tensor_reduce(out=pair[:, 0:1], in_=lnT, axis=mybir.AxisListType.X,
                            op=ALU.max)
    negln = main.tile([P, CL], F32, tag="negln", name="negln")
    nc.vector.tensor_scalar(out=negln, in0=lnT, scalar1=-1.0, scalar2=None, op0=ALU.mult)
    nc.vector.tensor_reduce(out=pair[:, 1:2], in_=negln, axis=mybir.AxisListType.X,
                            op=ALU.max)
    cc_in = dram.tile([1, 2], F32, name="cc_in")
    cc_out = dram.tile([1, 2], F32, name="cc_out")
    nc.sync.dma_start(out=cc_in, in_=pair[0:1, :])
    nc.gpsimd.collective_compute(
        "AllReduce", ALU.max,
        replica_groups=[list(range(NCORES))],
        ins=[cc_in[:].opt()], outs=[cc_out[:].opt()],
    )
    mx = main.tile([P, 2], F32, tag="mx", name="mx")
    cc_out_ap = cc_out[:]
    mx_b = bass.AP(tensor=cc_out_ap.tensor, offset=cc_out_ap.offset,
                   ap=[[0, P], cc_out_ap.ap[1]])
    nc.sync.dma_start(out=mx, in_=mx_b)

    # drate = sigmoid(5 - 10*(ln - lnmin)/(lnmax - lnmin))
    den = main.tile([P, 1], F32, tag="den", name="den")
    nc.vector.tensor_add(den, mx[:, 0:1], mx[:, 1:2])
    rden = main.tile([P, 1], F32, tag="rden", name="rden")
    nc.vector.reciprocal(out=rden, in_=den)
    num = main.tile([P, CL], F32, tag="num", name="num")
    nc.vector.tensor_add(num, lnT, _bc(mx[:, 1:2], CL))
    nc.vector.tensor_mul(num, num, _bc(rden, CL))
    norm = main.tile([P, CL], F32, tag="norm", name="norm")
    nc.vector.tensor_scalar(out=norm, in0=num, scalar1=-10.0, scalar2=5.0,
                            op0=ALU.mult, op1=ALU.add)
    drate = main.tile([P, CL], F32, tag="drate", name="drate")
    nc.scalar.activation(out=drate, in_=norm, func=AF.Sigmoid)
    ndrate = main.tile([P, CL], F32, tag="ndrate", name="ndrate")
    nc.vector.tensor_scalar(out=ndrate, in0=drate, scalar1=-1.0, scalar2=None,
                            op0=ALU.mult)
    hardf = main.tile([P, CL], U8, tag="hardf", name="hardf")
    nc.vector.tensor_tensor(out=hardf, in0=scal_sb[:, :, 1], in1=drate, op=ALU.is_gt)

    # folded weight scalars: w = target*A2 + B2, then -0.5 for the sign trick
    am1 = main.tile([P, CL], F32, tag="am1", name="am1")
    nc.vector.tensor_scalar(out=am1, in0=alpha, scalar1=-1.0, scalar2=None, op0=ALU.add)
    Aw = main.tile([P, CL], F32, tag="Aw", name="Aw")
    nc.vector.tensor_scalar(out=Aw, in0=beta, scalar1=-1.0, scalar2=1.0,
                            op0=ALU.mult, op1=ALU.add)
    nc.vector.copy_predicated(out=Aw, mask=hardf, data=am1)
    ones = main.tile([P, CL], F32, tag="ones", name="ones")
    nc.vector.memset(ones, 1.0)
    Bw = main.tile([P, CL], F32, tag="Bw", name="Bw")
    nc.vector.tensor_copy(Bw, beta)
    nc.vector.copy_predicated(out=Bw, mask=hardf, data=ones)
    A2 = main.tile([P, CL], F32, tag="A2", name="A2")
    nc.vector.tensor_mul(A2, u2, Aw)
    B2s = main.tile([P, CL], F32, tag="B2s", name="B2s")
    nc.vector.tensor_mul(B2s, posgt, Aw)
    nc.vector.tensor_add(B2s, B2s, Bw)
    nc.vector.tensor_scalar(out=B2s, in0=B2s, scalar1=-0.5, scalar2=None, op0=ALU.add)

    # ---------------- bisection for per-class k-th smallest ----------------
    loE = main.tile([P, CL], F32, tag="loE", name="loE")
    nc.vector.memset(loE, 0.0)
    thr = main.tile([P, CL], F32, tag="thr", name="thr")
    # ACT classes (0..2): stat = sum sign(te-gmb) = 2cnt-B -> thr = 2k-B;
    # DVE classes (3..4): stat = cnt -> thr = k. flag_hi = stat > thr.
    nc.vector.tensor_copy(thr, kq)
    nc.vector.tensor_scalar(out=thr[:, 0:2], in0=kq[:, 0:2], scalar1=2.0, scalar2=-BF,
                            op0=ALU.mult, op1=ALU.add)
    bacc_t = main.tile([P, CL], F32, tag="bacc", name="bacc")
    te = main.tile([P, CL], F32, tag="te", name="te")
    flo_f = main.tile([P, CL], F32, tag="flo_f", name="flo_f")

    for it in range(BIS_ITERS):
        w_half = 2.0 ** -(it + 1)
        nc.vector.tensor_scalar(out=te, in0=loE, scalar1=w_half, scalar2=None,
                                op0=ALU.add)
        for c in range(2):
            nc.scalar.activation(out=junkA, in_=gqT[c], func=AF.Sign,
                                 scale=-1.0, bias=te[:, c : c + 1],
                                 accum_out=bacc_t[:, c : c + 1])
        for c in range(2, CL):
            nc.vector.tensor_scalar(out=junkD, in0=gqT[c], scalar1=te[:, c : c + 1],
                                    scalar2=None, op0=ALU.is_lt, op1=ALU.add,
                                    accum_out=bacc_t[:, c : c + 1])
        bRp = repl_reduce(bacc_t, f"bR{it}")
        nc.vector.tensor_tensor(out=flo_f, in0=bRp, in1=thr, op=ALU.is_le)
        nc.vector.scalar_tensor_tensor(out=loE, in0=flo_f, scalar=w_half, in1=loE,
                                       op0=ALU.mult, op1=ALU.add)

    # final threshold (already offset); hard classes get sentinel -9
    tadj = main.tile([P, CL], F32, tag="tadj", name="tadj")
    nc.vector.tensor_scalar(out=tadj, in0=loE, scalar1=2.0 ** -(BIS_ITERS + 1),
                            scalar2=None, op0=ALU.add)
    tstar = main.tile([P, CL], F32, tag="tstar", name="tstar")
    nc.vector.tensor_copy(tstar, tadj)
    neg9 = main.tile([P, CL], F32, tag="neg9", name="neg9")
    nc.vector.memset(neg9, -9.0)
    nc.vector.copy_predicated(out=tadj, mask=hardf, data=neg9)
    ntadj = main.tile([P, CL], F32, tag="ntadj", name="ntadj")
    nc.vector.tensor_scalar(out=ntadj, in0=tadj, scalar1=-1.0, scalar2=None,
                            op0=ALU.mult)

    # ---------------- final weighted sum ----------------
    # m1 = bce*(1 - di*ix) with di = (rand>drate), ix = (g>=0.8);
    # total_c = sum(m1*w1) + 0.5*sum(m1*sign(gq-tadj))
    f1acc = main.tile([P, CL], F32, tag="f1acc", name="f1acc")
    f2acc = main.tile([P, CL], F32, tag="f2acc", name="f2acc")
    m1T = []
    for c in range(CL):
        sF = tmp3.tile([P, N], F32, tag="t2", name="sF")
        nc.scalar.activation(out=sF, in_=randT[c], func=AF.Sign,
                             bias=ndrate[:, c : c + 1])
        six = tmp3.tile([P, N], F32, tag="t3", name="six")
        nc.scalar.activation(out=six, in_=gT[c], func=AF.Sign, bias=n08_col)
        ssum = tmp3.tile([P, N], F32, tag="t4", name="ssum")
        nc.vector.tensor_add(ssum, sF, six)
        m1 = tmp3.tile([P, N], F32, tag="t6", name="m1", bufs=6)
        nc.vector.scalar_tensor_tensor(out=m1, in0=ssum, scalar=1.9, in1=bceT[c],
                                       op0=ALU.is_lt, op1=ALU.mult)
        m1T.append(m1)
        w1 = tmp3.tile([P, N], F32, tag="t1", name="w1")
        nc.scalar.activation(out=w1, in_=targT[c], func=AF.Identity,
                             scale=A2[:, c : c + 1], bias=B2s[:, c : c + 1])
        nc.vector.scalar_tensor_tensor(out=junkD, in0=m1, scalar=1.0, in1=w1,
                                       op0=ALU.mult, op1=ALU.mult,
                                       accum_out=f1acc[:, c : c + 1])
    for c in range(CL):
        sg = tmp3.tile([P, N], F32, tag="t0", name="sg")
        nc.scalar.activation(out=sg, in_=gqT[c], func=AF.Sign,
                             bias=ntadj[:, c : c + 1])
        nc.vector.scalar_tensor_tensor(out=junkD, in0=m1T[c], scalar=1.0, in1=sg,
                                       op0=ALU.mult, op1=ALU.mult,
                                       accum_out=f2acc[:, c : c + 1])

    diff = main.tile([P, CL], F32, tag="diff", name="diff")
    nc.vector.scalar_tensor_tensor(out=diff, in0=f2acc, scalar=0.5, in1=f1acc,
                                   op0=ALU.mult, op1=ALU.add)
    # total = sum over partitions and classes via two tiny matmuls
    pT = psum.tile([CL, 1], F32, tag="p1", name="pT")
    nc.tensor.matmul(out=pT, lhsT=diff, rhs=ones_col, start=True, stop=True)
    s5 = main.tile([CL, 1], F32, tag="s5", name="s5")
    nc.vector.tensor_copy(s5, pT)
    pS = psum.tile([1, 1], F32, tag="p2", name="pS")
    nc.tensor.matmul(out=pS, lhsT=s5, rhs=ones5, start=True, stop=True)

    outt = main.tile([1, NOUT], F32, tag="outt", name="outt")
    nc.vector.memset(outt, 0.0)
    nc.vector.tensor_copy(outt[0:1, 0:1], pS)
    # debug slots
    nc.vector.tensor_copy(outt[0:1, 1 : 1 + CL], tstar[0:1, :])
    nc.vector.tensor_copy(outt[0:1, 6 : 6 + CL], kq[0:1, :])
    nc.vector.tensor_copy(outt[0:1, 11 : 11 + CL], possum[0:1, :])
    nc.vector.tensor_copy(outt[0:1, 21 : 21 + CL], drate[0:1, :])
    nc.vector.tensor_copy(outt[0:1, 26 : 26 + CL], hardf[0:1, :])
    nc.sync.dma_start(out=out_d[:, :], in_=outt)
    ctx.close()


def build_nc():
    nc = bacc.Bacc("TRN2", target_bir_lowering=False, debug=False,
                   num_devices=NCORES)
    pred_d = nc.dram_tensor("pred", [CL, B], F32, kind="ExternalInput")
    targ_d = nc.dram_tensor("target", [CL, B], F32, kind="ExternalInput")
    rand_d = nc.dram_tensor("rand", [CL, B], F32, kind="ExternalInput")
    scal_d = nc.dram_tensor("scal", [CL, 2], F32, kind="ExternalInput")
    out_d = nc.dram_tensor("out", [1, NOUT], F32, kind="ExternalOutput")
    with tile.TileContext(nc) as tc:
        _body(tc, pred_d, targ_d, rand_d, scal_d, out_d)
    nc.compile()
    return nc


def _shard(pred, target, rand_mat, hard_rand, pos_prop):
    in_maps = []
    for i in range(NCORES):
        sl = slice(CL * i, CL * (i + 1))
        in_maps.append({
            "pred": np.ascontiguousarray(pred[:, sl].T),
            "target": np.ascontiguousarray(target[:, sl].T),
            "rand": np.ascontiguousarray(rand_mat[:, sl].T),
            "scal": np.ascontiguousarray(
                np.stack([pos_prop[sl], hard_rand[0, sl]], axis=1)
            ).astype(np.float32),
        })
    return in_maps


def _gather(res):
    total = sum(float(r["out"][0, 0]) for r in res.results)
    return np.array(total / (B * C), dtype=np.float32)


def kernel(pred, target, rand_mat, hard_rand, pos_prop):
    if "nc" not in _CACHED:
        _CACHED["nc"] = build_nc()
    nc = _CACHED["nc"]
    in_maps = _shard(pred, target, rand_mat, hard_rand, pos_prop)
    # the first exec after a fresh NEFF load occasionally dies with a
    # transient NRT_EXEC_UNIT_UNRECOVERABLE; one retry recovers it
    try:
        res = run_bass_kernel_spmd(nc, in_maps, core_ids=list(range(NCORES)))
    except Exception:
        import time as _time
        _time.sleep(2.0)
        res = run_bass_kernel_spmd(nc, in_maps, core_ids=list(range(NCORES)))
    return _gather(res)


def _install_ntff_shim():
    """The agent image's antenv lacks axon_hooks; shim it from trn_boot."""
    import sys, types
    try:
        from antenv import axon_hooks  # noqa: F401
        return
    except ImportError:
        pass
    import antenv
    from trn_agent_boot.trn_boot import _ntff_profile_via_ctypes
    hook = _ntff_profile_via_ctypes("/opt/axon/libaxon_pjrt.so")
    mod = types.ModuleType("antenv.axon_hooks")
    mod.get_axon_ntff_profile_hook = lambda: hook
    mod.set_axon_ntff_profile_hook = lambda h: None
    sys.modules["antenv.axon_hooks"] = mod
    antenv.axon_hooks = mod
    # artifact upload has no cloud access here; keep traces local
    import concourse.bass_utils as bu
    bu.upload_artifacts = lambda tmpdir: tmpdir


def kernel_profiled(pred, target, rand_mat, hard_rand, pos_prop, **trace_kwargs):
    """Same as kernel() but with NTFF tracing; returns (out, BassKernelResults)."""
    _install_ntff_shim()
    if "nc" not in _CACHED:
        _CACHED["nc"] = build_nc()
    nc = _CACHED["nc"]
    in_maps = _shard(pred, target, rand_mat, hard_rand, pos_prop)
    res = run_bass_kernel_spmd(nc, in_maps, core_ids=list(range(NCORES)),
                               trace=True, **trace_kwargs)
    return _gather(res), res
